# revision 37
# baseline (speedup 1.0000x reference)
"""Causal multi-head self-attention with RoPE on 8 Trainium2 NeuronCores.

Sharding: batch (4) x query-half (2) -> 8 cores, no collectives.
Each core computes full K/V for its batch; query rows are split between the
two cores of a batch in a causally-balanced schedule (4 slots of 256 rows
with 4/8/12/16 key-blocks each, ascending).  Causal masking is multiplicative
mask input data, so one SPMD program serves both halves.

Single fused pipeline, bf16 end-to-end (fp32 PSUM accumulation):
  per 512-seq chunk st: K^T proj+RoPE, V proj (+ones col), Q^T proj+RoPE,
  then attention slot st (which needs exactly k-blocks 0..4*(st+1)-1).
  Output projection for all slots is deferred to the end so it overlaps the
  ACT-bound tail of the last (deepest) attention slot, and so Wo can reuse
  Wk's SBUF space.

Layouts are transposed [feature, seq] so no on-device transposes are needed:
  K^T/Q^T = W^T.T @ X^T            (per 128-row head pair)
  RoPE    = cos*x + sin*(P@x)      (P = constant pair-rotation matrix)
  S^T     = Krot^T.T-slice @ Qrot^T  (keys on partitions; softmax along
                                      partitions via the ones-row trick)
  scores for both heads of a pair go into one [128,2,256] PSUM tile so a
  single Exp activation covers 512 elements (halves ACT instruction count)
  exp     = ACT Exp(scale=1/8) -> bf16
  A^T,l   = [V|1].T-free matmul accumulated over key blocks in PSUM
  denom broadcast = gpsimd partition_broadcast (keeps PE/ACT/DVE free)
  out     = A^T.T @ Wo^T           (natural [seq, feature] output layout)
"""

import os
import sys
import math

if "/opt/trn_rl_repo" not in sys.path:
    sys.path.append("/opt/trn_rl_repo")

import numpy as np
import ml_dtypes

import concourse.bass as bass
import concourse.tile as tile
from concourse import bacc, mybir
from concourse.bass_utils import run_bass_kernel_spmd

B = 4
S = 2048
D = 1024
H = 16
DK = 64
THETA = 10000.0

NEP = H // 2          # head pairs (128-partition groups)
QT = 256              # query tile width (free dim of score matmuls)
KB = 128              # key block (partition dim of score output)
NSLOT = 4             # query slots per core
CNT = [4, 8, 12, 16]  # k-blocks per slot (uniform across cores, ascending)
TILES_J = [[1, 3, 5, 7], [0, 2, 4, 6]]  # 256-row q-tile indices per half
VW = DK + 1           # V columns per head incl. trailing ones column

F32R = mybir.dt.float32r
F32 = mybir.dt.float32
BF16 = mybir.dt.bfloat16

_cache = {}

NO_GPSIMD = bool(int(os.environ.get("KERNEL_NO_GPSIMD", "1")))


def _build_program():
    if "nc" in _cache:
        return _cache["nc"]

    nc = bacc.Bacc("TRN2")

    xt_d = nc.dram_tensor("xt", [D, S], BF16, kind="ExternalInput")
    xq_d = nc.dram_tensor("xq", [D, NSLOT * QT], BF16, kind="ExternalInput")
    wkt_d = nc.dram_tensor("wkt", [D, D], BF16, kind="ExternalInput")
    wvt_d = nc.dram_tensor("wvt", [D, D], BF16, kind="ExternalInput")
    wqt_d = nc.dram_tensor("wqt", [D, D], BF16, kind="ExternalInput")
    wot_d = nc.dram_tensor("wot", [D, D], BF16, kind="ExternalInput")
    cosk_d = nc.dram_tensor("cosk", [128, S], BF16, kind="ExternalInput")
    sink_d = nc.dram_tensor("sink", [128, S], BF16, kind="ExternalInput")
    cosq_d = nc.dram_tensor("cosq", [128, NSLOT * QT], BF16, kind="ExternalInput")
    sinq_d = nc.dram_tensor("sinq", [128, NSLOT * QT], BF16, kind="ExternalInput")
    mask_d = nc.dram_tensor("mask", [128, 4, 2, QT], BF16, kind="ExternalInput")
    permt_d = nc.dram_tensor("permt", [128, 128], BF16, kind="ExternalInput")
    y_d = nc.dram_tensor("y", [NSLOT * QT, D], F32, kind="ExternalOutput")

    xt_t = xt_d.rearrange("(n p) s -> p n s", p=128)
    xq_t = xq_d.rearrange("(n p) s -> p n s", p=128)
    wkt_t = wkt_d.rearrange("(n p) e -> p n e", p=128)
    wqt_t = wqt_d.rearrange("(n p) e -> p n e", p=128)

    with tile.TileContext(nc) as tc:
        with (
            tc.tile_pool(name="wgt", bufs=1) as wgt,
            tc.tile_pool(name="kvq", bufs=1) as kvq,
            tc.tile_pool(name="tab", bufs=1) as tab,
            tc.tile_pool(name="xsp", bufs=2) as xsp,
            tc.tile_pool(name="xqp", bufs=1) as xqp,
            tc.tile_pool(name="wrk", bufs=2) as wrk,
            tc.tile_pool(name="epl", bufs=3) as epl,
            tc.tile_pool(name="atp", bufs=1) as atp,
            tc.tile_pool(name="nrm", bufs=2) as nrm,
            tc.tile_pool(name="ocp", bufs=2) as ocp,
            tc.tile_pool(name="ps", bufs=1, space="PSUM") as ps,
            tc.tile_pool(name="psa", bufs=2, space="PSUM") as psa,
        ):
            # ---- persistent SBUF tiles ----
            permt = wgt.tile([128, 128], BF16, tag="permt", bufs=1)
            wk = [wgt.tile([128, 8, 128], BF16, tag=f"wk{e}", bufs=1,
                           name=f"wk{e}") for e in range(NEP)]
            wq = [wgt.tile([128, 8, 128], BF16, tag=f"wq{e}", bufs=1,
                           name=f"wq{e}") for e in range(NEP)]
            wv = [wgt.tile([128, D], BF16, tag=f"wv{d}", bufs=1,
                           name=f"wv{d}") for d in range(8)]
            krot = [kvq.tile([128, S], BF16, tag=f"krot{e}", bufs=1,
                             name=f"krot{e}") for e in range(NEP)]
            vt = [kvq.tile([128, H * VW], BF16, tag=f"vt{k}", bufs=1,
                           name=f"vt{k}") for k in range(S // KB)]
            qrot = [kvq.tile([128, NSLOT * QT], BF16, tag=f"qrot{e}", bufs=1,
                             name=f"qrot{e}") for e in range(NEP)]
            cosk = tab.tile([128, S], BF16, tag="cosk", bufs=1)
            sink = tab.tile([128, S], BF16, tag="sink", bufs=1)
            cosq = tab.tile([128, NSLOT * QT], BF16, tag="cosq", bufs=1)
            sinq = tab.tile([128, NSLOT * QT], BF16, tag="sinq", bufs=1)
            masks = tab.tile([128, 4, 2, QT], BF16, tag="masks", bufs=1)

            # ---- input DMAs, in priority order ----
            for dd in range(4):
                nc.sync.dma_start(wk[0][:, 2 * dd:2 * dd + 2, :],
                                  wkt_t[:, 2 * dd:2 * dd + 2, 0:128])
            xs_first = xsp.tile([128, 8, 512], BF16, tag="xs", bufs=2,
                                name="xs_first")
            nc.gpsimd.dma_start(xs_first[:], xt_t[:, :, 0:512])
            xq_first = xqp.tile([128, 8, QT], BF16, tag="xq", bufs=1,
                                name="xq_first")
            nc.gpsimd.dma_start(xq_first[:], xq_t[:, :, 0:QT])
            for e in range(1, NEP):
                nc.sync.dma_start(wk[e][:], wkt_t[:, :, e * 128:(e + 1) * 128])
            nc.sync.dma_start(cosk[:], cosk_d[:])
            nc.sync.dma_start(sink[:], sink_d[:])
            nc.sync.dma_start(permt[:], permt_d[:])
            for d in range(8):
                nc.sync.dma_start(wv[d][:], wvt_d[d * 128:(d + 1) * 128, :])
            for e in range(NEP):
                nc.sync.dma_start(wq[e][:], wqt_t[:, :, e * 128:(e + 1) * 128])
            nc.sync.dma_start(cosq[:], cosq_d[:])
            nc.sync.dma_start(sinq[:], sinq_d[:])
            nc.sync.dma_start(masks[:], mask_d[:])

            # ones columns of vt (denominator rows for the AV matmul)
            if NO_GPSIMD:
                ones_t = tab.tile([VW, DK], F32, tag="ones", bufs=1)
                nc.vector.memset(ones_t[DK:VW, :], 1.0)
            for k in range(S // KB):
                nc.vector.memset(
                    vt[k].rearrange("p (h w) -> p h w", w=VW)[:, :, DK], 1.0
                )

            # ---- helpers ----
            def k_rope(kraw, e, csl):
                pp = ps.tile([128, 512], F32, tag="perm", bufs=1, name="ppk")
                nc.tensor.matmul(pp[:], permt[:], kraw[:], start=True, stop=True)
                t_c = wrk.tile([128, 512], BF16, tag="t_c", bufs=2, name="t_c")
                nc.vector.tensor_mul(t_c[:], kraw[:], cosk[:, csl])
                t_s = wrk.tile([128, 512], BF16, tag="t_s", bufs=2, name="t_s")
                nc.vector.tensor_mul(t_s[:], pp[:], sink[:, csl])
                nc.vector.tensor_add(krot[e][:, csl], t_c[:], t_s[:])

            def q_rope(qraw, e, csl):
                pp = ps.tile([128, QT], F32, tag="perm", bufs=1, name="ppq")
                nc.tensor.matmul(pp[:], permt[:], qraw[:], start=True, stop=True)
                t_c = wrk.tile([128, QT], BF16, tag="qt_c", bufs=2, name="qt_c")
                nc.vector.tensor_mul(t_c[:], qraw[:], cosq[:, csl])
                t_s = wrk.tile([128, QT], BF16, tag="qt_s", bufs=2, name="qt_s")
                nc.vector.tensor_mul(t_s[:], pp[:], sinq[:, csl])
                nc.vector.tensor_add(qrot[e][:, csl], t_c[:], t_s[:])

            aT = [[None] * NEP for _ in range(NSLOT)]

            def attention_slot(sl, fillers=None):
                C = CNT[sl]
                qsl = slice(sl * QT, (sl + 1) * QT)
                for e in range(NEP):
                    acc = psa.tile([VW, 2, QT], F32, tag="acc", bufs=2,
                                   name="acc")
                    pend = []

                    def flush_av(kb, ex):
                        # one accumulation group for the whole bank: start
                        # zeroes the 2KB zero-region (both h slices), stop on
                        # the very last matmul touching it
                        for h in range(2):
                            hh = 2 * e + h
                            nc.tensor.matmul(
                                acc[:, h, :],
                                vt[kb][:, hh * VW:(hh + 1) * VW],
                                ex[h][:],
                                start=(kb == 0 and h == 0),
                                stop=(kb == C - 1 and h == 1),
                            )

                    for kb in range(C):
                        exs = []
                        for h in range(2):
                            pb = h * DK
                            psc = ps.tile([128, QT], F32, tag="sc", bufs=2,
                                          name="psc")
                            nc.tensor.matmul(
                                psc[:],
                                krot[e][pb:pb + DK, kb * KB:(kb + 1) * KB],
                                qrot[e][pb:pb + DK, qsl],
                                start=True, stop=True,
                                tile_position=(pb, 0),
                            )
                            exh = epl.tile([128, QT], BF16, tag="ex", bufs=6,
                                           name="exh")
                            nc.scalar.activation(
                                exh[:], psc[:],
                                mybir.ActivationFunctionType.Exp,
                                scale=1.0 / math.sqrt(DK),
                            )
                            if kb >= C - 4:
                                em = epl.tile([128, QT], BF16, tag="em",
                                              bufs=4, name="em")
                                nc.vector.tensor_mul(
                                    em[:], exh[:],
                                    masks[:, kb - (C - 4), h, :]
                                )
                                exh = em
                            exs.append(exh)
                        pend.append((kb, exs))
                        if len(pend) > 2:
                            flush_av(*pend.pop(0))
                    for p_ in pend:
                        flush_av(*p_)

                    # softmax normalization: 1/denom, broadcast, scale
                    a = atp.tile([128, QT], BF16, tag=f"aT{sl}_{e}", bufs=1,
                                 name=f"aT{sl}_{e}")
                    aT[sl][e] = a
                    lrow = nrm.tile([VW, 2, QT], F32R, tag="lrow", bufs=1,
                                    name="lrow")
                    with nc.allow_low_precision(
                        reason="f32r tile holds full f32 bits"
                    ):
                        nc.vector.reciprocal(
                            lrow[DK:VW, :, :], acc[DK:VW, :, :]
                        )
                    if NO_GPSIMD:
                        pb = ps.tile([DK, 2, QT], F32, tag="rbp", bufs=1,
                                     name="pb")
                        nc.tensor.matmul(pb[:],
                                         ones_t.bitcast(F32R)[DK:VW, :],
                                         lrow[DK:VW, :, :],
                                         start=True, stop=True)
                        rb = nrm.tile([DK, 2, QT], F32, tag="rb", bufs=2,
                                      name="rb")
                        nc.vector.tensor_copy(rb[:], pb[:])
                    else:
                        rb = nrm.tile([DK, 2, QT], F32R, tag="rb", bufs=1,
                                      name="rb")
                        nc.gpsimd.partition_broadcast(rb[:], lrow[DK:VW, :, :])
                    nc.vector.tensor_mul(a[0:DK, :], acc[0:DK, 0, :],
                                         rb[:, 0, :])
                    tmp = nrm.tile([DK, QT], BF16, tag="tmp", bufs=2,
                                   name="tmp")
                    nc.vector.tensor_mul(tmp[:], acc[0:DK, 1, :], rb[:, 1, :])
                    nc.sync.dma_start(a[DK:128, :], tmp[:])
                    if fillers and e >= 2:
                        for _ in range(2):
                            if fillers:
                                fillers.pop(0)()

            def oproj_group(sl, qs, et):
                def emit():
                    po = ps.tile([128, 512], F32, tag="proj", bufs=2,
                                 name="po")
                    for d in range(8):
                        nc.tensor.matmul(
                            po[:],
                            aT[sl][d][:, qs * 128:(qs + 1) * 128],
                            wo[d][:, et * 512:(et + 1) * 512],
                            start=(d == 0), stop=(d == 7),
                        )
                    for eh in range(2):
                        ot = ocp.tile([128, QT], F32, tag="ot", bufs=4,
                                      name="ot")
                        nc.vector.tensor_copy(ot[:], po[:, eh * QT:(eh + 1) * QT])
                        nc.sync.dma_start(
                            y_d[sl * QT + qs * 128:sl * QT + (qs + 1) * 128,
                                et * 512 + eh * QT:et * 512 + (eh + 1) * QT],
                            ot[:],
                        )
                return emit

            # ---- fused projection + attention pipeline ----
            xs_tiles = [xs_first]
            xq_tiles = [xq_first]
            for st in range(1, 4):
                # prefetches are issued at the top of each chunk below
                xs_tiles.append(None)
                xq_tiles.append(None)

            for st in range(4):
                if st > 0:
                    xs = xsp.tile([128, 8, 512], BF16, tag="xs", bufs=2,
                                  name="xs")
                    nc.gpsimd.dma_start(xs[:], xt_t[:, :, st * 512:(st + 1) * 512])
                    xq = xqp.tile([128, 8, QT], BF16, tag="xq", bufs=1,
                                  name="xq")
                    nc.gpsimd.dma_start(xq[:], xq_t[:, :, st * QT:(st + 1) * QT])
                else:
                    xs, xq = xs_first, xq_first

                # K projection + RoPE for seq columns [512*st, 512*(st+1))
                pend = []
                for e in range(NEP):
                    pk = ps.tile([128, 512], F32, tag="proj", bufs=2,
                                 name="pk")
                    for d in range(8):
                        nc.tensor.matmul(
                            pk[:], wk[e][:, d, :], xs[:, d, :],
                            start=(d == 0), stop=(d == 7),
                        )
                    kraw = wrk.tile([128, 512], BF16, tag="kraw", bufs=2,
                                    name="kraw")
                    nc.vector.tensor_copy(kraw[:], pk[:])
                    pend.append((kraw, e, slice(st * 512, (st + 1) * 512)))
                    if len(pend) > 2:
                        k_rope(*pend.pop(0))

                # V projection for k-blocks 4*st .. 4*st+3
                for half in range(4):
                    kb = 4 * st + half
                    off = half * KB
                    for et in range(2):
                        pv = ps.tile([128, 512], F32, tag="proj", bufs=2,
                                     name="pv")
                        for d in range(8):
                            nc.tensor.matmul(
                                pv[:], xs[:, d, off:off + KB],
                                wv[d][:, et * 512:(et + 1) * 512],
                                start=(d == 0), stop=(d == 7),
                            )
                        dst = vt[kb].rearrange("p (h w) -> p h w", w=VW)
                        if et == 0:
                            nc.scalar.copy(
                                dst[:, 0:8, 0:DK],
                                pv[:].rearrange("p (h w) -> p h w", w=DK),
                            )
                        else:
                            nc.vector.tensor_copy(
                                dst[:, 8:16, 0:DK],
                                pv[:].rearrange("p (h w) -> p h w", w=DK),
                            )
                    if len(pend) > 1:
                        k_rope(*pend.pop(0))

                # Q projection + RoPE for slot st
                pendq = []
                for e in range(NEP):
                    pq = ps.tile([128, QT], F32, tag="proj", bufs=2,
                                 name="pq")
                    for d in range(8):
                        nc.tensor.matmul(
                            pq[:], wq[e][:, d, :], xq[:, d, :],
                            start=(d == 0), stop=(d == 7),
                        )
                    qraw = wrk.tile([128, QT], BF16, tag="qraw", bufs=2,
                                    name="qraw")
                    nc.vector.tensor_copy(qraw[:], pq[:])
                    pendq.append((qraw, e, slice(st * QT, (st + 1) * QT)))
                    if len(pendq) > 2:
                        q_rope(*pendq.pop(0))
                for p_ in pend:
                    k_rope(*p_)
                for p_ in pendq:
                    q_rope(*p_)

                if st == 3:
                    # Wo reuses Wk's SBUF slots (tag ring); DMA starts once
                    # chunk 3's K projection has consumed wk, and overlaps
                    # the deepest attention slot below.
                    wo = [wgt.tile([128, D], BF16, tag=f"wk{e}", bufs=1,
                                   name=f"wo{e}") for e in range(NEP)]
                    for e in range(NEP):
                        nc.sync.dma_start(wo[e][:],
                                          wot_d[e * 128:(e + 1) * 128, :])

                if st < 3:
                    attention_slot(st)
                else:
                    # slots 0-2's output projections fill PE while slot 3's
                    # deeper softmax keeps the scalar engine saturated
                    fillers = [oproj_group(sl, qs, et)
                               for sl in range(3)
                               for qs in range(2) for et in range(2)]
                    attention_slot(3, fillers)
                    for f in fillers:
                        f()

            # ---- slot 3 output projection (needs all of slot 3's aT) ----
            for qs in range(2):
                for et in range(2):
                    oproj_group(3, qs, et)()

    nc.compile()
    nc.finalize()
    _cache["nc"] = nc
    return nc


def _rope_tables(pos):
    """cos/sin tables in [128, n] head-pair layout (row e -> pair (e%64)//2)."""
    k = np.arange(DK // 2, dtype=np.float32)
    inv_freq = (THETA ** (-2.0 * k / DK)).astype(np.float32)
    ang = inv_freq[:, None] * pos.astype(np.float32)[None, :]  # [32, n]
    cos64 = np.repeat(np.cos(ang), 2, axis=0)
    sin64 = np.repeat(np.sin(ang), 2, axis=0)
    cos = np.concatenate([cos64, cos64], axis=0)
    sin = np.concatenate([sin64, sin64], axis=0)
    return (np.ascontiguousarray(cos).astype(ml_dtypes.bfloat16),
            np.ascontiguousarray(sin).astype(ml_dtypes.bfloat16))


def _masks(j):
    """[128, 4, 2, QT] bf16 multiplicative causal masks for half j.

    Slot-independent: for slot sl (C = CNT[sl]) the mask applies to the last
    four k-blocks C-4..C-1.  j=0 owns tiles 2t+1 -> [1, 1, triA, triB];
    j=1 owns tiles 2t -> [triA, triB, 0, 0].
    """
    p = np.arange(KB)[:, None]
    f = np.arange(QT)[None, :]
    triA = (f >= p).astype(np.float32)
    triB = (f >= p + KB).astype(np.float32)
    ones = np.ones((KB, QT), np.float32)
    zeros = np.zeros((KB, QT), np.float32)
    blocks = [ones, ones, triA, triB] if j == 0 else [triA, triB, zeros, zeros]
    m = np.stack([np.stack([blk] * 2, axis=0) for blk in blocks], axis=0)
    # [4, 2, KB, QT] -> [KB, 4, 2, QT]
    return np.ascontiguousarray(m.transpose(2, 0, 1, 3)).astype(
        ml_dtypes.bfloat16)


def _host_inputs(in_features, token_positions, Wq, Wk, Wv, Wo):
    X = np.asarray(in_features, dtype=np.float32)
    pos = np.asarray(token_positions)
    bf = ml_dtypes.bfloat16
    wqt = np.ascontiguousarray(np.asarray(Wq, np.float32).T).astype(bf)
    wkt = np.ascontiguousarray(np.asarray(Wk, np.float32).T).astype(bf)
    wvt = np.ascontiguousarray(np.asarray(Wv, np.float32).T).astype(bf)
    wot = np.ascontiguousarray(np.asarray(Wo, np.float32).T).astype(bf)
    cosk, sink = _rope_tables(pos)

    permt = np.zeros((128, 128), np.float32)
    for i in range(64):
        permt[2 * i + 1, 2 * i] = -1.0
        permt[2 * i, 2 * i + 1] = 1.0
    permt = permt.astype(bf)

    in_maps = []
    for core in range(8):
        b, j = core // 2, core % 2
        rows = np.concatenate(
            [np.arange(t * QT, (t + 1) * QT) for t in TILES_J[j]])
        cosq, sinq = _rope_tables(pos[rows])
        in_maps.append({
            "xt": np.ascontiguousarray(X[b].T).astype(bf),
            "xq": np.ascontiguousarray(X[b][rows].T).astype(bf),
            "wkt": wkt, "wvt": wvt, "wqt": wqt, "wot": wot,
            "cosk": cosk, "sink": sink, "cosq": cosq, "sinq": sinq,
            "mask": _masks(j), "permt": permt,
        })
    return in_maps


def kernel(in_features, token_positions, Wq, Wk, Wv, Wo):
    nc = _build_program()
    in_maps = _host_inputs(in_features, token_positions, Wq, Wk, Wv, Wo)

    trace = bool(int(os.environ.get("KERNEL_TRACE", "0")))
    res = run_bass_kernel_spmd(nc, in_maps, core_ids=list(range(8)), trace=trace)
    kernel.last_result = res

    out = np.empty((B, S, D), np.float32)
    for core in range(8):
        b, j = core // 2, core % 2
        y = res.results[core]["y"]
        for s_i, t in enumerate(TILES_J[j]):
            out[b, t * QT:(t + 1) * QT, :] = y[s_i * QT:(s_i + 1) * QT, :]
    return out


# revision 42
# speedup vs baseline: 1.0490x; 1.0490x over previous
"""Causal multi-head self-attention with RoPE on 8 Trainium2 NeuronCores.

Sharding: batch (4) x query-half (2) -> 8 cores, no collectives.
Each core computes full K/V for its batch; query rows are split between the
two cores of a batch in a causally-balanced schedule (4 slots of 256 rows
with 4/8/12/16 key-blocks each, ascending).  Causal masking is multiplicative
mask input data, so one SPMD program serves both halves.

Single fused pipeline, bf16 end-to-end (fp32 PSUM accumulation):
  per 512-seq chunk st: K^T proj+RoPE, V proj (+ones col), Q^T proj+RoPE,
  then attention slot st (which needs exactly k-blocks 0..4*(st+1)-1).
  Output projection for all slots is deferred to the end so it overlaps the
  ACT-bound tail of the last (deepest) attention slot, and so Wo can reuse
  Wk's SBUF space.

Layouts are transposed [feature, seq] so no on-device transposes are needed:
  K^T/Q^T = W^T.T @ X^T            (per 128-row head pair)
  RoPE    = cos*x + sin*(P@x)      (P = constant pair-rotation matrix)
  S^T     = Krot^T.T-slice @ Qrot^T  (keys on partitions; softmax along
                                      partitions via the ones-row trick)
  scores for both heads of a pair go into one [128,2,256] PSUM tile so a
  single Exp activation covers 512 elements (halves ACT instruction count)
  exp     = ACT Exp(scale=1/8) -> bf16
  A^T,l   = [V|1].T-free matmul accumulated over key blocks in PSUM
  denom broadcast = gpsimd partition_broadcast (keeps PE/ACT/DVE free)
  out     = A^T.T @ Wo^T           (natural [seq, feature] output layout)
"""

import os
import sys
import math

if "/opt/trn_rl_repo" not in sys.path:
    sys.path.append("/opt/trn_rl_repo")

import numpy as np
import ml_dtypes

import concourse.bass as bass
import concourse.tile as tile
from concourse import bacc, mybir
from concourse.bass_utils import run_bass_kernel_spmd

B = 4
S = 2048
D = 1024
H = 16
DK = 64
THETA = 10000.0

NEP = H // 2          # head pairs (128-partition groups)
QT = 256              # query tile width (free dim of score matmuls)
KB = 128              # key block (partition dim of score output)
NSLOT = 4             # query slots per core
CNT = [4, 8, 12, 16]  # k-blocks per slot (uniform across cores, ascending)
TILES_J = [[1, 3, 5, 7], [0, 2, 4, 6]]  # 256-row q-tile indices per half
VW = DK + 1           # V columns per head incl. trailing ones column

F32R = mybir.dt.float32r
F32 = mybir.dt.float32
BF16 = mybir.dt.bfloat16

_cache = {}

NO_GPSIMD = bool(int(os.environ.get("KERNEL_NO_GPSIMD", "1")))


def _build_program():
    if "nc" in _cache:
        return _cache["nc"]

    nc = bacc.Bacc("TRN2")

    xt_d = nc.dram_tensor("xt", [D, S], BF16, kind="ExternalInput")
    xq_d = nc.dram_tensor("xq", [D, NSLOT * QT], BF16, kind="ExternalInput")
    wkt_d = nc.dram_tensor("wkt", [D, D], BF16, kind="ExternalInput")
    wvt_d = nc.dram_tensor("wvt", [D, D], BF16, kind="ExternalInput")
    wqt_d = nc.dram_tensor("wqt", [D, D], BF16, kind="ExternalInput")
    wot_d = nc.dram_tensor("wot", [D, D], BF16, kind="ExternalInput")
    cosk_d = nc.dram_tensor("cosk", [128, S], BF16, kind="ExternalInput")
    sink_d = nc.dram_tensor("sink", [128, S], BF16, kind="ExternalInput")
    cosq_d = nc.dram_tensor("cosq", [128, NSLOT * QT], BF16, kind="ExternalInput")
    sinq_d = nc.dram_tensor("sinq", [128, NSLOT * QT], BF16, kind="ExternalInput")
    mask_d = nc.dram_tensor("mask", [128, 4, 2, QT], BF16, kind="ExternalInput")
    permt_d = nc.dram_tensor("permt", [128, 128], BF16, kind="ExternalInput")
    y_d = nc.dram_tensor("y", [NSLOT * QT, D], F32, kind="ExternalOutput")

    xt_t = xt_d.rearrange("(n p) s -> p n s", p=128)
    xq_t = xq_d.rearrange("(n p) s -> p n s", p=128)
    wkt_t = wkt_d.rearrange("(n p) e -> p n e", p=128)
    wqt_t = wqt_d.rearrange("(n p) e -> p n e", p=128)

    with tile.TileContext(nc) as tc:
        with (
            tc.tile_pool(name="wgt", bufs=1) as wgt,
            tc.tile_pool(name="kvq", bufs=1) as kvq,
            tc.tile_pool(name="tab", bufs=1) as tab,
            tc.tile_pool(name="xsp", bufs=2) as xsp,
            tc.tile_pool(name="xqp", bufs=1) as xqp,
            tc.tile_pool(name="wrk", bufs=2) as wrk,
            tc.tile_pool(name="epl", bufs=3) as epl,
            tc.tile_pool(name="atp", bufs=1) as atp,
            tc.tile_pool(name="nrm", bufs=2) as nrm,
            tc.tile_pool(name="ocp", bufs=2) as ocp,
            tc.tile_pool(name="ps", bufs=1, space="PSUM") as ps,
            tc.tile_pool(name="psa", bufs=2, space="PSUM") as psa,
        ):
            # ---- persistent SBUF tiles ----
            permt = wgt.tile([128, 128], BF16, tag="permt", bufs=1)
            wk = [wgt.tile([128, 8, 128], BF16, tag=f"wk{e}", bufs=1,
                           name=f"wk{e}") for e in range(NEP)]
            wq = [wgt.tile([128, 8, 128], BF16, tag=f"wq{e}", bufs=1,
                           name=f"wq{e}") for e in range(NEP)]
            wv = [wgt.tile([128, D], BF16, tag=f"wv{d}", bufs=1,
                           name=f"wv{d}") for d in range(8)]
            krot = [kvq.tile([128, S], BF16, tag=f"krot{e}", bufs=1,
                             name=f"krot{e}") for e in range(NEP)]
            vt = [kvq.tile([128, H * VW], BF16, tag=f"vt{k}", bufs=1,
                           name=f"vt{k}") for k in range(S // KB)]
            qrot = [kvq.tile([128, NSLOT * QT], BF16, tag=f"qrot{e}", bufs=1,
                             name=f"qrot{e}") for e in range(NEP)]
            cosk = tab.tile([128, S], BF16, tag="cosk", bufs=1)
            sink = tab.tile([128, S], BF16, tag="sink", bufs=1)
            cosq = tab.tile([128, NSLOT * QT], BF16, tag="cosq", bufs=1)
            sinq = tab.tile([128, NSLOT * QT], BF16, tag="sinq", bufs=1)
            masks = tab.tile([128, 4, 2, QT], BF16, tag="masks", bufs=1)

            # ---- input DMAs, in priority order ----
            for dd in range(4):
                nc.sync.dma_start(wk[0][:, 2 * dd:2 * dd + 2, :],
                                  wkt_t[:, 2 * dd:2 * dd + 2, 0:128])
            xs_first = xsp.tile([128, 8, 512], BF16, tag="xs", bufs=2,
                                name="xs_first")
            nc.gpsimd.dma_start(xs_first[:], xt_t[:, :, 0:512])
            xq_first = xqp.tile([128, 8, QT], BF16, tag="xq", bufs=1,
                                name="xq_first")
            nc.gpsimd.dma_start(xq_first[:], xq_t[:, :, 0:QT])
            for e in range(1, NEP):
                nc.sync.dma_start(wk[e][:], wkt_t[:, :, e * 128:(e + 1) * 128])
            nc.sync.dma_start(cosk[:], cosk_d[:])
            nc.sync.dma_start(sink[:], sink_d[:])
            nc.sync.dma_start(permt[:], permt_d[:])
            for d in range(8):
                nc.sync.dma_start(wv[d][:], wvt_d[d * 128:(d + 1) * 128, :])
            for e in range(NEP):
                nc.sync.dma_start(wq[e][:], wqt_t[:, :, e * 128:(e + 1) * 128])
            nc.sync.dma_start(cosq[:], cosq_d[:])
            nc.sync.dma_start(sinq[:], sinq_d[:])
            nc.sync.dma_start(masks[:], mask_d[:])

            # ones columns of vt (denominator rows for the AV matmul)
            if NO_GPSIMD:
                ones_t = tab.tile([VW, DK], F32, tag="ones", bufs=1)
                nc.vector.memset(ones_t[DK:VW, :], 1.0)
            for k in range(S // KB):
                nc.vector.memset(
                    vt[k].rearrange("p (h w) -> p h w", w=VW)[:, :, DK], 1.0
                )

            # ---- helpers ----
            def k_rope(kraw, e, csl):
                pp = ps.tile([128, 512], F32, tag="perm", bufs=1, name="ppk")
                nc.tensor.matmul(pp[:], permt[:], kraw[:], start=True, stop=True)
                t_c = wrk.tile([128, 512], BF16, tag="t_c", bufs=2, name="t_c")
                nc.vector.tensor_mul(t_c[:], kraw[:], cosk[:, csl])
                t_s = wrk.tile([128, 512], BF16, tag="t_s", bufs=2, name="t_s")
                nc.vector.tensor_mul(t_s[:], pp[:], sink[:, csl])
                nc.vector.tensor_add(krot[e][:, csl], t_c[:], t_s[:])

            def q_rope(qraw, e, csl):
                pp = ps.tile([128, QT], F32, tag="perm", bufs=1, name="ppq")
                nc.tensor.matmul(pp[:], permt[:], qraw[:], start=True, stop=True)
                t_c = wrk.tile([128, QT], BF16, tag="qt_c", bufs=2, name="qt_c")
                nc.vector.tensor_mul(t_c[:], qraw[:], cosq[:, csl])
                t_s = wrk.tile([128, QT], BF16, tag="qt_s", bufs=2, name="qt_s")
                nc.vector.tensor_mul(t_s[:], pp[:], sinq[:, csl])
                nc.vector.tensor_add(qrot[e][:, csl], t_c[:], t_s[:])

            aT = [[None] * NEP for _ in range(NSLOT)]

            def normalize(sl, e, acc):
                a = atp.tile([128, QT], BF16, tag=f"aT{sl}_{e}", bufs=1,
                             name=f"aT{sl}_{e}")
                aT[sl][e] = a
                lrow = nrm.tile([VW, 2, QT], F32R, tag="lrow", bufs=1,
                                name="lrow")
                with nc.allow_low_precision(
                    reason="f32r tile holds full f32 bits"
                ):
                    nc.vector.reciprocal(
                        lrow[DK:VW, :, :], acc[DK:VW, :, :]
                    )
                pb = ps.tile([DK, 2, QT], F32, tag="rbp", bufs=1,
                             name="pb")
                nc.tensor.matmul(pb[:],
                                 ones_t.bitcast(F32R)[DK:VW, :],
                                 lrow[DK:VW, :, :],
                                 start=True, stop=True)
                rb = nrm.tile([DK, 2, QT], F32, tag="rb", bufs=1,
                              name="rb")
                nc.vector.tensor_copy(rb[:], pb[:])
                nc.vector.tensor_mul(a[0:DK, :], acc[0:DK, 0, :],
                                     rb[:, 0, :])
                tmp = nrm.tile([DK, QT], BF16, tag="tmp", bufs=2,
                               name="tmp")
                nc.vector.tensor_mul(tmp[:], acc[0:DK, 1, :], rb[:, 1, :])
                nc.sync.dma_start(a[DK:128, :], tmp[:])

            def attention_pair(sl_a, sl_b, fillers=None):
                # slots sl_a, sl_b (= sl_a+1) are contiguous in qrot; for
                # k-blocks both need, ONE N=512 score matmul + ONE wide exp
                # covers both.  Cb-4 == Ca, so solo tiles are exactly the
                # masked tail of sl_b, and joint tiles only mask sl_a's tail.
                Ca, Cb = CNT[sl_a], CNT[sl_b]
                qsl_ab = slice(sl_a * QT, (sl_b + 1) * QT)
                qsl_b = slice(sl_b * QT, (sl_b + 1) * QT)
                for e in range(NEP):
                    acc_a = psa.tile([VW, 2, QT], F32, tag="acc", bufs=2,
                                     name="acc_a")
                    acc_b = psa.tile([VW, 2, QT], F32, tag="acc", bufs=2,
                                     name="acc_b")
                    pend = []

                    def flush_av(kb, exa, exb):
                        for h in range(2):
                            hh = 2 * e + h
                            vs = vt[kb][:, hh * VW:(hh + 1) * VW]
                            if exa is not None:
                                nc.tensor.matmul(
                                    acc_a[:, h, :], vs, exa[h],
                                    start=(kb == 0 and h == 0),
                                    stop=(kb == Ca - 1 and h == 1),
                                )
                            nc.tensor.matmul(
                                acc_b[:, h, :], vs, exb[h],
                                start=(kb == 0 and h == 0),
                                stop=(kb == Cb - 1 and h == 1),
                            )

                    for kb in range(Cb):
                        joint = kb < Ca
                        exa = [] if joint else None
                        exb = []
                        for h in range(2):
                            pb_ = h * DK
                            krs = krot[e][pb_:pb_ + DK,
                                          kb * KB:(kb + 1) * KB]
                            if joint:
                                psc = ps.tile([128, 2 * QT], F32, tag="sc",
                                              bufs=2, name="psc")
                                nc.tensor.matmul(
                                    psc[:], krs,
                                    qrot[e][pb_:pb_ + DK, qsl_ab],
                                    start=True, stop=True,
                                    tile_position=(pb_, 0),
                                )
                                ex = epl.tile([128, 2 * QT], BF16, tag="ex",
                                              bufs=4, name="ex")
                                nc.scalar.activation(
                                    ex[:], psc[:],
                                    mybir.ActivationFunctionType.Exp,
                                    scale=1.0 / math.sqrt(DK),
                                )
                                ea = ex[:, 0:QT]
                                if kb >= Ca - 4:
                                    em = epl.tile([128, QT], BF16, tag="em",
                                                  bufs=4, name="em")
                                    nc.vector.tensor_mul(
                                        em[:], ex[:, 0:QT],
                                        masks[:, kb - (Ca - 4), h, :]
                                    )
                                    ea = em[:]
                                exa.append(ea)
                                exb.append(ex[:, QT:2 * QT])
                            else:
                                psc = ps.tile([128, QT], F32, tag="sc",
                                              bufs=2, name="psc")
                                nc.tensor.matmul(
                                    psc[:], krs,
                                    qrot[e][pb_:pb_ + DK, qsl_b],
                                    start=True, stop=True,
                                    tile_position=(pb_, 0),
                                )
                                exh = epl.tile([128, QT], BF16, tag="exs",
                                               bufs=3, name="exh")
                                nc.scalar.activation(
                                    exh[:], psc[:],
                                    mybir.ActivationFunctionType.Exp,
                                    scale=1.0 / math.sqrt(DK),
                                )
                                em = epl.tile([128, QT], BF16, tag="em",
                                              bufs=4, name="em")
                                nc.vector.tensor_mul(
                                    em[:], exh[:],
                                    masks[:, kb - (Cb - 4), h, :]
                                )
                                exb.append(em[:])
                        pend.append((kb, exa, exb))
                        if len(pend) > 2:
                            flush_av(*pend.pop(0))
                    for p_ in pend:
                        flush_av(*p_)

                    normalize(sl_a, e, acc_a)
                    normalize(sl_b, e, acc_b)
                    if fillers and e >= 2:
                        for _ in range(2):
                            if fillers:
                                fillers.pop(0)()

            def oproj_group(sl, qs, et):
                def emit():
                    po = ps.tile([128, 512], F32, tag="proj", bufs=2,
                                 name="po")
                    for d in range(8):
                        nc.tensor.matmul(
                            po[:],
                            aT[sl][d][:, qs * 128:(qs + 1) * 128],
                            wo[d][:, et * 512:(et + 1) * 512],
                            start=(d == 0), stop=(d == 7),
                        )
                    for eh in range(2):
                        ot = ocp.tile([128, QT], F32, tag="ot", bufs=4,
                                      name="ot")
                        nc.vector.tensor_copy(ot[:], po[:, eh * QT:(eh + 1) * QT])
                        nc.sync.dma_start(
                            y_d[sl * QT + qs * 128:sl * QT + (qs + 1) * 128,
                                et * 512 + eh * QT:et * 512 + (eh + 1) * QT],
                            ot[:],
                        )
                return emit

            # ---- fused projection + attention pipeline ----
            xs_tiles = [xs_first]
            xq_tiles = [xq_first]
            for st in range(1, 4):
                # prefetches are issued at the top of each chunk below
                xs_tiles.append(None)
                xq_tiles.append(None)

            for st in range(4):
                if st > 0:
                    xs = xsp.tile([128, 8, 512], BF16, tag="xs", bufs=2,
                                  name="xs")
                    nc.gpsimd.dma_start(xs[:], xt_t[:, :, st * 512:(st + 1) * 512])
                    xq = xqp.tile([128, 8, QT], BF16, tag="xq", bufs=1,
                                  name="xq")
                    nc.gpsimd.dma_start(xq[:], xq_t[:, :, st * QT:(st + 1) * QT])
                else:
                    xs, xq = xs_first, xq_first

                # K projection + RoPE for seq columns [512*st, 512*(st+1))
                pend = []
                for e in range(NEP):
                    pk = ps.tile([128, 512], F32, tag="proj", bufs=2,
                                 name="pk")
                    for d in range(8):
                        nc.tensor.matmul(
                            pk[:], wk[e][:, d, :], xs[:, d, :],
                            start=(d == 0), stop=(d == 7),
                        )
                    kraw = wrk.tile([128, 512], BF16, tag="kraw", bufs=2,
                                    name="kraw")
                    nc.vector.tensor_copy(kraw[:], pk[:])
                    pend.append((kraw, e, slice(st * 512, (st + 1) * 512)))
                    if len(pend) > 2:
                        k_rope(*pend.pop(0))

                # V projection for k-blocks 4*st .. 4*st+3
                for half in range(4):
                    kb = 4 * st + half
                    off = half * KB
                    for et in range(2):
                        pv = ps.tile([128, 512], F32, tag="proj", bufs=2,
                                     name="pv")
                        for d in range(8):
                            nc.tensor.matmul(
                                pv[:], xs[:, d, off:off + KB],
                                wv[d][:, et * 512:(et + 1) * 512],
                                start=(d == 0), stop=(d == 7),
                            )
                        dst = vt[kb].rearrange("p (h w) -> p h w", w=VW)
                        if et == 0:
                            nc.scalar.copy(
                                dst[:, 0:8, 0:DK],
                                pv[:].rearrange("p (h w) -> p h w", w=DK),
                            )
                        else:
                            nc.vector.tensor_copy(
                                dst[:, 8:16, 0:DK],
                                pv[:].rearrange("p (h w) -> p h w", w=DK),
                            )
                    if len(pend) > 1:
                        k_rope(*pend.pop(0))

                # Q projection + RoPE for slot st
                pendq = []
                for e in range(NEP):
                    pq = ps.tile([128, QT], F32, tag="proj", bufs=2,
                                 name="pq")
                    for d in range(8):
                        nc.tensor.matmul(
                            pq[:], wq[e][:, d, :], xq[:, d, :],
                            start=(d == 0), stop=(d == 7),
                        )
                    qraw = wrk.tile([128, QT], BF16, tag="qraw", bufs=2,
                                    name="qraw")
                    nc.vector.tensor_copy(qraw[:], pq[:])
                    pendq.append((qraw, e, slice(st * QT, (st + 1) * QT)))
                    if len(pendq) > 2:
                        q_rope(*pendq.pop(0))
                for p_ in pend:
                    k_rope(*p_)
                for p_ in pendq:
                    q_rope(*p_)

                if st == 3:
                    # Wo reuses Wk's SBUF slots (tag ring); DMA starts once
                    # chunk 3's K projection has consumed wk, and overlaps
                    # the deepest attention slot below.
                    wo = [wgt.tile([128, D], BF16, tag=f"wk{e}", bufs=1,
                                   name=f"wo{e}") for e in range(NEP)]
                    for e in range(NEP):
                        nc.sync.dma_start(wo[e][:],
                                          wot_d[e * 128:(e + 1) * 128, :])

                if st == 1:
                    attention_pair(0, 1)
                elif st == 3:
                    # slots 0/1's output projections fill PE while the deep
                    # pair's softmax keeps the scalar engine saturated
                    fillers = [oproj_group(sl, qs, et)
                               for sl in range(2)
                               for qs in range(2) for et in range(2)]
                    attention_pair(2, 3, fillers)
                    for f in fillers:
                        f()

            # ---- slots 2/3 output projection ----
            for sl in (2, 3):
                for qs in range(2):
                    for et in range(2):
                        oproj_group(sl, qs, et)()

    nc.compile()
    nc.finalize()
    _cache["nc"] = nc
    return nc


def _rope_tables(pos):
    """cos/sin tables in [128, n] head-pair layout (row e -> pair (e%64)//2)."""
    k = np.arange(DK // 2, dtype=np.float32)
    inv_freq = (THETA ** (-2.0 * k / DK)).astype(np.float32)
    ang = inv_freq[:, None] * pos.astype(np.float32)[None, :]  # [32, n]
    cos64 = np.repeat(np.cos(ang), 2, axis=0)
    sin64 = np.repeat(np.sin(ang), 2, axis=0)
    cos = np.concatenate([cos64, cos64], axis=0)
    sin = np.concatenate([sin64, sin64], axis=0)
    return (np.ascontiguousarray(cos).astype(ml_dtypes.bfloat16),
            np.ascontiguousarray(sin).astype(ml_dtypes.bfloat16))


def _masks(j):
    """[128, 4, 2, QT] bf16 multiplicative causal masks for half j.

    Slot-independent: for slot sl (C = CNT[sl]) the mask applies to the last
    four k-blocks C-4..C-1.  j=0 owns tiles 2t+1 -> [1, 1, triA, triB];
    j=1 owns tiles 2t -> [triA, triB, 0, 0].
    """
    p = np.arange(KB)[:, None]
    f = np.arange(QT)[None, :]
    triA = (f >= p).astype(np.float32)
    triB = (f >= p + KB).astype(np.float32)
    ones = np.ones((KB, QT), np.float32)
    zeros = np.zeros((KB, QT), np.float32)
    blocks = [ones, ones, triA, triB] if j == 0 else [triA, triB, zeros, zeros]
    m = np.stack([np.stack([blk] * 2, axis=0) for blk in blocks], axis=0)
    # [4, 2, KB, QT] -> [KB, 4, 2, QT]
    return np.ascontiguousarray(m.transpose(2, 0, 1, 3)).astype(
        ml_dtypes.bfloat16)


def _host_inputs(in_features, token_positions, Wq, Wk, Wv, Wo):
    X = np.asarray(in_features, dtype=np.float32)
    pos = np.asarray(token_positions)
    bf = ml_dtypes.bfloat16
    wqt = np.ascontiguousarray(np.asarray(Wq, np.float32).T).astype(bf)
    wkt = np.ascontiguousarray(np.asarray(Wk, np.float32).T).astype(bf)
    wvt = np.ascontiguousarray(np.asarray(Wv, np.float32).T).astype(bf)
    wot = np.ascontiguousarray(np.asarray(Wo, np.float32).T).astype(bf)
    cosk, sink = _rope_tables(pos)

    permt = np.zeros((128, 128), np.float32)
    for i in range(64):
        permt[2 * i + 1, 2 * i] = -1.0
        permt[2 * i, 2 * i + 1] = 1.0
    permt = permt.astype(bf)

    in_maps = []
    for core in range(8):
        b, j = core // 2, core % 2
        rows = np.concatenate(
            [np.arange(t * QT, (t + 1) * QT) for t in TILES_J[j]])
        cosq, sinq = _rope_tables(pos[rows])
        in_maps.append({
            "xt": np.ascontiguousarray(X[b].T).astype(bf),
            "xq": np.ascontiguousarray(X[b][rows].T).astype(bf),
            "wkt": wkt, "wvt": wvt, "wqt": wqt, "wot": wot,
            "cosk": cosk, "sink": sink, "cosq": cosq, "sinq": sinq,
            "mask": _masks(j), "permt": permt,
        })
    return in_maps


def kernel(in_features, token_positions, Wq, Wk, Wv, Wo):
    nc = _build_program()
    in_maps = _host_inputs(in_features, token_positions, Wq, Wk, Wv, Wo)

    trace = bool(int(os.environ.get("KERNEL_TRACE", "0")))
    res = run_bass_kernel_spmd(nc, in_maps, core_ids=list(range(8)), trace=trace)
    kernel.last_result = res

    out = np.empty((B, S, D), np.float32)
    for core in range(8):
        b, j = core // 2, core % 2
        y = res.results[core]["y"]
        for s_i, t in enumerate(TILES_J[j]):
            out[b, t * QT:(t + 1) * QT, :] = y[s_i * QT:(s_i + 1) * QT, :]
    return out


# revision 53
# speedup vs baseline: 1.0746x; 1.0245x over previous
"""Causal multi-head self-attention with RoPE on 8 Trainium2 NeuronCores.

Sharding: batch (4) x query-half (2) -> 8 cores, no collectives.
Each core computes full K/V for its batch; query rows are split between the
two cores of a batch in a causally-balanced schedule (4 slots of 256 rows
with 4/8/12/16 key-blocks each, ascending).  Causal masking is multiplicative
mask input data, so one SPMD program serves both halves.

Single fused pipeline, bf16 end-to-end (fp32 PSUM accumulation):
  per 512-seq chunk st: K^T proj+RoPE, V proj (+ones col), Q^T proj+RoPE,
  then attention slot st (which needs exactly k-blocks 0..4*(st+1)-1).
  Output projection for all slots is deferred to the end so it overlaps the
  ACT-bound tail of the last (deepest) attention slot, and so Wo can reuse
  Wk's SBUF space.

Layouts are transposed [feature, seq] so no on-device transposes are needed:
  K^T/Q^T = W^T.T @ X^T            (per 128-row head pair)
  RoPE    = cos*x + sin*(P@x)      (P = constant pair-rotation matrix)
  S^T     = Krot^T.T-slice @ Qrot^T  (keys on partitions; softmax along
                                      partitions via the ones-row trick)
  scores for both heads of a pair go into one [128,2,256] PSUM tile so a
  single Exp activation covers 512 elements (halves ACT instruction count)
  exp     = ACT Exp(scale=1/8) -> bf16
  A^T,l   = [V|1].T-free matmul accumulated over key blocks in PSUM
  denom broadcast = gpsimd partition_broadcast (keeps PE/ACT/DVE free)
  out     = A^T.T @ Wo^T           (natural [seq, feature] output layout)
"""

import os
import sys
import math

if "/opt/trn_rl_repo" not in sys.path:
    sys.path.append("/opt/trn_rl_repo")

import numpy as np
import ml_dtypes

import concourse.bass as bass
import concourse.tile as tile
from concourse import bacc, mybir
from concourse.bass_utils import run_bass_kernel_spmd

B = 4
S = 2048
D = 1024
H = 16
DK = 64
THETA = 10000.0

NEP = H // 2          # head pairs (128-partition groups)
QT = 256              # query tile width (free dim of score matmuls)
KB = 128              # key block (partition dim of score output)
NSLOT = 4             # query slots per core
CNT = [4, 8, 12, 16]  # k-blocks per slot (uniform across cores, ascending)
TILES_J = [[1, 3, 5, 7], [0, 2, 4, 6]]  # 256-row q-tile indices per half
VW = DK + 1           # V columns per head incl. trailing ones column

F32R = mybir.dt.float32r
F32 = mybir.dt.float32
BF16 = mybir.dt.bfloat16

_cache = {}

NO_GPSIMD = bool(int(os.environ.get("KERNEL_NO_GPSIMD", "1")))


def _build_program():
    if "nc" in _cache:
        return _cache["nc"]

    nc = bacc.Bacc("TRN2")

    xt_d = nc.dram_tensor("xt", [D, S], BF16, kind="ExternalInput")
    xq_d = nc.dram_tensor("xq", [D, NSLOT * QT], BF16, kind="ExternalInput")
    wkt_d = nc.dram_tensor("wkt", [D, D], BF16, kind="ExternalInput")
    wvt_d = nc.dram_tensor("wvt", [D, D], BF16, kind="ExternalInput")
    wqt_d = nc.dram_tensor("wqt", [D, D], BF16, kind="ExternalInput")
    wot_d = nc.dram_tensor("wot", [D, D], BF16, kind="ExternalInput")
    cosk_d = nc.dram_tensor("cosk", [128, S], BF16, kind="ExternalInput")
    sink_d = nc.dram_tensor("sink", [128, S], BF16, kind="ExternalInput")
    cosq_d = nc.dram_tensor("cosq", [128, NSLOT * QT], BF16, kind="ExternalInput")
    sinq_d = nc.dram_tensor("sinq", [128, NSLOT * QT], BF16, kind="ExternalInput")
    mask_d = nc.dram_tensor("mask", [128, 4, 2, QT], BF16, kind="ExternalInput")
    permt_d = nc.dram_tensor("permt", [128, 128], BF16, kind="ExternalInput")
    y_d = nc.dram_tensor("y", [NSLOT * QT, D], F32, kind="ExternalOutput")

    xt_t = xt_d.rearrange("(n p) s -> p n s", p=128)
    xq_t = xq_d.rearrange("(n p) s -> p n s", p=128)
    wkt_t = wkt_d.rearrange("(n p) e -> p n e", p=128)
    wqt_t = wqt_d.rearrange("(n p) e -> p n e", p=128)

    with tile.TileContext(nc) as tc:
        with (
            tc.tile_pool(name="wgt", bufs=1) as wgt,
            tc.tile_pool(name="kvq", bufs=1) as kvq,
            tc.tile_pool(name="tab", bufs=1) as tab,
            tc.tile_pool(name="xsp", bufs=2) as xsp,
            tc.tile_pool(name="xqp", bufs=1) as xqp,
            tc.tile_pool(name="wrk", bufs=2) as wrk,
            tc.tile_pool(name="epl", bufs=3) as epl,
            tc.tile_pool(name="atp", bufs=1) as atp,
            tc.tile_pool(name="nrm", bufs=2) as nrm,
            tc.tile_pool(name="ocp", bufs=2) as ocp,
            tc.tile_pool(name="ps", bufs=1, space="PSUM") as ps,
            tc.tile_pool(name="psa", bufs=2, space="PSUM") as psa,
        ):
            # ---- persistent SBUF tiles ----
            permt = wgt.tile([128, 128], BF16, tag="permt", bufs=1)
            wk = [wgt.tile([128, 8, 128], BF16, tag=f"wk{e}", bufs=1,
                           name=f"wk{e}") for e in range(NEP)]
            wq = [wgt.tile([128, 8, 128], BF16, tag=f"wq{e}", bufs=1,
                           name=f"wq{e}") for e in range(NEP)]
            wv = [wgt.tile([128, D], BF16, tag=f"wv{d}", bufs=1,
                           name=f"wv{d}") for d in range(8)]
            krot = [kvq.tile([128, S], BF16, tag=f"krot{e}", bufs=1,
                             name=f"krot{e}") for e in range(NEP)]
            vt = [kvq.tile([128, H * VW], BF16, tag=f"vt{k}", bufs=1,
                           name=f"vt{k}") for k in range(S // KB)]
            qrot = [kvq.tile([128, NSLOT * QT], BF16, tag=f"qrot{e}", bufs=1,
                             name=f"qrot{e}") for e in range(NEP)]
            cosk = tab.tile([128, S], BF16, tag="cosk", bufs=1)
            sink = tab.tile([128, S], BF16, tag="sink", bufs=1)
            cosq = tab.tile([128, NSLOT * QT], BF16, tag="cosq", bufs=1)
            sinq = tab.tile([128, NSLOT * QT], BF16, tag="sinq", bufs=1)
            masks = tab.tile([128, 4, 2, QT], BF16, tag="masks", bufs=1)

            # ---- input DMAs, in priority order ----
            for dd in range(4):
                nc.sync.dma_start(wk[0][:, 2 * dd:2 * dd + 2, :],
                                  wkt_t[:, 2 * dd:2 * dd + 2, 0:128])
            xs_first = xsp.tile([128, 8, 512], BF16, tag="xs", bufs=2,
                                name="xs_first")
            nc.gpsimd.dma_start(xs_first[:], xt_t[:, :, 0:512])
            xq_first = xqp.tile([128, 8, QT], BF16, tag="xq", bufs=1,
                                name="xq_first")
            nc.gpsimd.dma_start(xq_first[:], xq_t[:, :, 0:QT])
            for e in range(1, NEP):
                nc.sync.dma_start(wk[e][:], wkt_t[:, :, e * 128:(e + 1) * 128])
            nc.sync.dma_start(cosk[:], cosk_d[:])
            nc.sync.dma_start(sink[:], sink_d[:])
            nc.sync.dma_start(permt[:], permt_d[:])
            for d in range(8):
                nc.sync.dma_start(wv[d][:], wvt_d[d * 128:(d + 1) * 128, :])
            for e in range(NEP):
                nc.sync.dma_start(wq[e][:], wqt_t[:, :, e * 128:(e + 1) * 128])
            nc.sync.dma_start(cosq[:], cosq_d[:])
            nc.sync.dma_start(sinq[:], sinq_d[:])
            nc.sync.dma_start(masks[:], mask_d[:])

            # ones columns of vt (denominator rows for the AV matmul)
            if NO_GPSIMD:
                ones_t = tab.tile([VW, DK], F32, tag="ones", bufs=1)
                nc.vector.memset(ones_t[DK:VW, :], 1.0)
            for k in range(S // KB):
                nc.vector.memset(
                    vt[k].rearrange("p (h w) -> p h w", w=VW)[:, :, DK], 1.0
                )

            # ---- helpers ----
            def k_rope(kraw, e, csl):
                pp = ps.tile([128, 512], F32, tag="perm", bufs=1, name="ppk")
                nc.tensor.matmul(pp[:], permt[:], kraw[:], start=True, stop=True)
                t_c = wrk.tile([128, 512], BF16, tag="t_c", bufs=2, name="t_c")
                nc.vector.tensor_mul(t_c[:], kraw[:], cosk[:, csl])
                t_s = wrk.tile([128, 512], BF16, tag="t_s", bufs=2, name="t_s")
                nc.vector.tensor_mul(t_s[:], pp[:], sink[:, csl])
                nc.vector.tensor_add(krot[e][:, csl], t_c[:], t_s[:])

            def q_rope(qraw, e, csl):
                pp = ps.tile([128, QT], F32, tag="perm", bufs=1, name="ppq")
                nc.tensor.matmul(pp[:], permt[:], qraw[:], start=True, stop=True)
                t_c = wrk.tile([128, QT], BF16, tag="qt_c", bufs=2, name="qt_c")
                nc.vector.tensor_mul(t_c[:], qraw[:], cosq[:, csl])
                t_s = wrk.tile([128, QT], BF16, tag="qt_s", bufs=2, name="qt_s")
                nc.vector.tensor_mul(t_s[:], pp[:], sinq[:, csl])
                nc.vector.tensor_add(qrot[e][:, csl], t_c[:], t_s[:])

            aT = [[None] * NEP for _ in range(NSLOT)]

            def normalize(sl, e, acc):
                a = atp.tile([128, QT], BF16, tag=f"aT{sl}_{e}", bufs=1,
                             name=f"aT{sl}_{e}")
                aT[sl][e] = a
                lrow = nrm.tile([VW, 2, QT], F32R, tag="lrow", bufs=1,
                                name="lrow")
                with nc.allow_low_precision(
                    reason="f32r tile holds full f32 bits"
                ):
                    nc.vector.reciprocal(
                        lrow[DK:VW, :, :], acc[DK:VW, :, :]
                    )
                pb = ps.tile([DK, 2, QT], F32, tag="rbp", bufs=1,
                             name="pb")
                nc.tensor.matmul(pb[:],
                                 ones_t.bitcast(F32R)[DK:VW, :],
                                 lrow[DK:VW, :, :],
                                 start=True, stop=True)
                rb = nrm.tile([DK, 2, QT], F32, tag="rb", bufs=1,
                              name="rb")
                nc.vector.tensor_copy(rb[:], pb[:])
                nc.vector.tensor_mul(a[0:DK, :], acc[0:DK, 0, :],
                                     rb[:, 0, :])
                tmp = nrm.tile([DK, QT], BF16, tag="tmp", bufs=2,
                               name="tmp")
                nc.vector.tensor_mul(tmp[:], acc[0:DK, 1, :], rb[:, 1, :])
                nc.sync.dma_start(a[DK:128, :], tmp[:])

            def attention_pair(sl_a, sl_b, fillers=None):
                # slots sl_a, sl_b (= sl_a+1) are contiguous in qrot; for
                # k-blocks both need, ONE N=512 score matmul + ONE wide exp
                # covers both.  Cb-4 == Ca, so solo tiles are exactly the
                # masked tail of sl_b, and joint tiles only mask sl_a's tail.
                Ca, Cb = CNT[sl_a], CNT[sl_b]
                qsl_ab = slice(sl_a * QT, (sl_b + 1) * QT)
                qsl_b = slice(sl_b * QT, (sl_b + 1) * QT)
                for e in range(NEP):
                    acc_a = psa.tile([VW, 2, QT], F32, tag="acc", bufs=2,
                                     name="acc_a")
                    acc_b = psa.tile([VW, 2, QT], F32, tag="acc", bufs=2,
                                     name="acc_b")
                    pend = []

                    def flush_av(kb, exa, exb):
                        for h in range(2):
                            hh = 2 * e + h
                            vs = vt[kb][:, hh * VW:(hh + 1) * VW]
                            if exa is not None:
                                nc.tensor.matmul(
                                    acc_a[:, h, :], vs, exa[h],
                                    start=(kb == 0 and h == 0),
                                    stop=(kb == Ca - 1 and h == 1),
                                )
                            nc.tensor.matmul(
                                acc_b[:, h, :], vs, exb[h],
                                start=(kb == 0 and h == 0),
                                stop=(kb == Cb - 1 and h == 1),
                            )

                    for kb in range(Cb):
                        joint = kb < Ca
                        exa = [] if joint else None
                        exb = []
                        for h in range(2):
                            pb_ = h * DK
                            krs = krot[e][pb_:pb_ + DK,
                                          kb * KB:(kb + 1) * KB]
                            if joint:
                                psc = ps.tile([128, 2 * QT], F32, tag="sc",
                                              bufs=2, name="psc")
                                nc.tensor.matmul(
                                    psc[:], krs,
                                    qrot[e][pb_:pb_ + DK, qsl_ab],
                                    start=True, stop=True,
                                    tile_position=(pb_, 0),
                                )
                                ex = epl.tile([128, 2 * QT], BF16, tag="ex",
                                              bufs=5, name="ex")
                                nc.scalar.activation(
                                    ex[:], psc[:],
                                    mybir.ActivationFunctionType.Exp,
                                    scale=1.0 / math.sqrt(DK),
                                )
                                ea = ex[:, 0:QT]
                                if kb >= Ca - 4:
                                    em = epl.tile([128, QT], BF16, tag="em",
                                                  bufs=4, name="em")
                                    nc.vector.tensor_mul(
                                        em[:], ex[:, 0:QT],
                                        masks[:, kb - (Ca - 4), h, :]
                                    )
                                    ea = em[:]
                                exa.append(ea)
                                exb.append(ex[:, QT:2 * QT])
                            else:
                                psc = ps.tile([128, QT], F32, tag="sc",
                                              bufs=2, name="psc")
                                nc.tensor.matmul(
                                    psc[:], krs,
                                    qrot[e][pb_:pb_ + DK, qsl_b],
                                    start=True, stop=True,
                                    tile_position=(pb_, 0),
                                )
                                exh = epl.tile([128, QT], BF16, tag="exs",
                                               bufs=3, name="exh")
                                nc.scalar.activation(
                                    exh[:], psc[:],
                                    mybir.ActivationFunctionType.Exp,
                                    scale=1.0 / math.sqrt(DK),
                                )
                                em = epl.tile([128, QT], BF16, tag="em",
                                              bufs=4, name="em")
                                nc.vector.tensor_mul(
                                    em[:], exh[:],
                                    masks[:, kb - (Cb - 4), h, :]
                                )
                                exb.append(em[:])
                        pend.append((kb, exa, exb))
                        if len(pend) > 3:
                            flush_av(*pend.pop(0))
                    for p_ in pend:
                        flush_av(*p_)

                    normalize(sl_a, e, acc_a)
                    normalize(sl_b, e, acc_b)
                    if fillers and e >= 2:
                        for _ in range(2):
                            if fillers:
                                fillers.pop(0)()

            def oproj_group(sl, qs, et):
                def emit():
                    po = ps.tile([128, 512], F32, tag="proj", bufs=2,
                                 name="po")
                    for d in range(8):
                        nc.tensor.matmul(
                            po[:],
                            aT[sl][d][:, qs * 128:(qs + 1) * 128],
                            wo[d][:, et * 512:(et + 1) * 512],
                            start=(d == 0), stop=(d == 7),
                        )
                    for eh in range(2):
                        ot = ocp.tile([128, QT], F32, tag="ot", bufs=4,
                                      name="ot")
                        nc.vector.tensor_copy(ot[:], po[:, eh * QT:(eh + 1) * QT])
                        nc.sync.dma_start(
                            y_d[sl * QT + qs * 128:sl * QT + (qs + 1) * 128,
                                et * 512 + eh * QT:et * 512 + (eh + 1) * QT],
                            ot[:],
                        )
                return emit

            # ---- fused projection + attention pipeline ----
            xs_tiles = [xs_first]
            xq_tiles = [xq_first]
            for st in range(1, 4):
                # prefetches are issued at the top of each chunk below
                xs_tiles.append(None)
                xq_tiles.append(None)

            for st in range(4):
                if st > 0:
                    xs = xsp.tile([128, 8, 512], BF16, tag="xs", bufs=2,
                                  name="xs")
                    nc.gpsimd.dma_start(xs[:], xt_t[:, :, st * 512:(st + 1) * 512])
                    xq = xqp.tile([128, 8, QT], BF16, tag="xq", bufs=1,
                                  name="xq")
                    nc.gpsimd.dma_start(xq[:], xq_t[:, :, st * QT:(st + 1) * QT])
                else:
                    xs, xq = xs_first, xq_first

                # K projection + RoPE for seq columns [512*st, 512*(st+1))
                pend = []
                for e in range(NEP):
                    pk = ps.tile([128, 512], F32, tag="proj", bufs=2,
                                 name="pk")
                    for d in range(8):
                        nc.tensor.matmul(
                            pk[:], wk[e][:, d, :], xs[:, d, :],
                            start=(d == 0), stop=(d == 7),
                        )
                    kraw = wrk.tile([128, 512], BF16, tag="kraw", bufs=2,
                                    name="kraw")
                    nc.scalar.copy(kraw[:], pk[:])
                    pend.append((kraw, e, slice(st * 512, (st + 1) * 512)))
                    if len(pend) > 2:
                        k_rope(*pend.pop(0))

                # V projection for k-blocks 4*st .. 4*st+3
                for half in range(4):
                    kb = 4 * st + half
                    off = half * KB
                    for et in range(2):
                        pv = ps.tile([128, 512], F32, tag="proj", bufs=2,
                                     name="pv")
                        for d in range(8):
                            nc.tensor.matmul(
                                pv[:], xs[:, d, off:off + KB],
                                wv[d][:, et * 512:(et + 1) * 512],
                                start=(d == 0), stop=(d == 7),
                            )
                        dst = vt[kb].rearrange("p (h w) -> p h w", w=VW)
                        if et == 0:
                            nc.scalar.copy(
                                dst[:, 0:8, 0:DK],
                                pv[:].rearrange("p (h w) -> p h w", w=DK),
                            )
                        else:
                            nc.scalar.copy(
                                dst[:, 8:16, 0:DK],
                                pv[:].rearrange("p (h w) -> p h w", w=DK),
                            )
                    if len(pend) > 1:
                        k_rope(*pend.pop(0))

                # Q projection + RoPE for slot st
                pendq = []
                for e in range(NEP):
                    pq = ps.tile([128, QT], F32, tag="proj", bufs=2,
                                 name="pq")
                    for d in range(8):
                        nc.tensor.matmul(
                            pq[:], wq[e][:, d, :], xq[:, d, :],
                            start=(d == 0), stop=(d == 7),
                        )
                    qraw = wrk.tile([128, QT], BF16, tag="qraw", bufs=2,
                                    name="qraw")
                    nc.vector.tensor_copy(qraw[:], pq[:])
                    pendq.append((qraw, e, slice(st * QT, (st + 1) * QT)))
                    if len(pendq) > 2:
                        q_rope(*pendq.pop(0))
                for p_ in pend:
                    k_rope(*p_)
                for p_ in pendq:
                    q_rope(*p_)

                if st == 3:
                    # Wo reuses Wk's SBUF slots (tag ring); DMA starts once
                    # chunk 3's K projection has consumed wk, and overlaps
                    # the deepest attention slot below.
                    wo = [wgt.tile([128, D], BF16, tag=f"wk{e}", bufs=1,
                                   name=f"wo{e}") for e in range(NEP)]
                    for e in range(NEP):
                        nc.sync.dma_start(wo[e][:],
                                          wot_d[e * 128:(e + 1) * 128, :])

                if st == 1:
                    attention_pair(0, 1)
                elif st == 3:
                    # slots 0/1's output projections fill PE while the deep
                    # pair's softmax keeps the scalar engine saturated
                    fillers = [oproj_group(sl, qs, et)
                               for sl in range(2)
                               for qs in range(2) for et in range(2)]
                    attention_pair(2, 3, fillers)
                    for f in fillers:
                        f()

            # ---- slots 2/3 output projection ----
            for sl in (2, 3):
                for qs in range(2):
                    for et in range(2):
                        oproj_group(sl, qs, et)()

    nc.compile()
    nc.finalize()
    _cache["nc"] = nc
    return nc


def _rope_tables(pos):
    """cos/sin tables in [128, n] head-pair layout (row e -> pair (e%64)//2)."""
    k = np.arange(DK // 2, dtype=np.float32)
    inv_freq = (THETA ** (-2.0 * k / DK)).astype(np.float32)
    ang = inv_freq[:, None] * pos.astype(np.float32)[None, :]  # [32, n]
    cos64 = np.repeat(np.cos(ang), 2, axis=0)
    sin64 = np.repeat(np.sin(ang), 2, axis=0)
    cos = np.concatenate([cos64, cos64], axis=0)
    sin = np.concatenate([sin64, sin64], axis=0)
    return (np.ascontiguousarray(cos).astype(ml_dtypes.bfloat16),
            np.ascontiguousarray(sin).astype(ml_dtypes.bfloat16))


def _masks(j):
    """[128, 4, 2, QT] bf16 multiplicative causal masks for half j.

    Slot-independent: for slot sl (C = CNT[sl]) the mask applies to the last
    four k-blocks C-4..C-1.  j=0 owns tiles 2t+1 -> [1, 1, triA, triB];
    j=1 owns tiles 2t -> [triA, triB, 0, 0].
    """
    p = np.arange(KB)[:, None]
    f = np.arange(QT)[None, :]
    triA = (f >= p).astype(np.float32)
    triB = (f >= p + KB).astype(np.float32)
    ones = np.ones((KB, QT), np.float32)
    zeros = np.zeros((KB, QT), np.float32)
    blocks = [ones, ones, triA, triB] if j == 0 else [triA, triB, zeros, zeros]
    m = np.stack([np.stack([blk] * 2, axis=0) for blk in blocks], axis=0)
    # [4, 2, KB, QT] -> [KB, 4, 2, QT]
    return np.ascontiguousarray(m.transpose(2, 0, 1, 3)).astype(
        ml_dtypes.bfloat16)


def _host_inputs(in_features, token_positions, Wq, Wk, Wv, Wo):
    X = np.asarray(in_features, dtype=np.float32)
    pos = np.asarray(token_positions)
    bf = ml_dtypes.bfloat16
    wqt = np.ascontiguousarray(np.asarray(Wq, np.float32).T).astype(bf)
    wkt = np.ascontiguousarray(np.asarray(Wk, np.float32).T).astype(bf)
    wvt = np.ascontiguousarray(np.asarray(Wv, np.float32).T).astype(bf)
    wot = np.ascontiguousarray(np.asarray(Wo, np.float32).T).astype(bf)
    cosk, sink = _rope_tables(pos)

    permt = np.zeros((128, 128), np.float32)
    for i in range(64):
        permt[2 * i + 1, 2 * i] = -1.0
        permt[2 * i, 2 * i + 1] = 1.0
    permt = permt.astype(bf)

    in_maps = []
    for core in range(8):
        b, j = core // 2, core % 2
        rows = np.concatenate(
            [np.arange(t * QT, (t + 1) * QT) for t in TILES_J[j]])
        cosq, sinq = _rope_tables(pos[rows])
        in_maps.append({
            "xt": np.ascontiguousarray(X[b].T).astype(bf),
            "xq": np.ascontiguousarray(X[b][rows].T).astype(bf),
            "wkt": wkt, "wvt": wvt, "wqt": wqt, "wot": wot,
            "cosk": cosk, "sink": sink, "cosq": cosq, "sinq": sinq,
            "mask": _masks(j), "permt": permt,
        })
    return in_maps


def kernel(in_features, token_positions, Wq, Wk, Wv, Wo):
    nc = _build_program()
    in_maps = _host_inputs(in_features, token_positions, Wq, Wk, Wv, Wo)

    trace = bool(int(os.environ.get("KERNEL_TRACE", "0")))
    res = run_bass_kernel_spmd(nc, in_maps, core_ids=list(range(8)), trace=trace)
    kernel.last_result = res

    out = np.empty((B, S, D), np.float32)
    for core in range(8):
        b, j = core // 2, core % 2
        y = res.results[core]["y"]
        for s_i, t in enumerate(TILES_J[j]):
            out[b, t * QT:(t + 1) * QT, :] = y[s_i * QT:(s_i + 1) * QT, :]
    return out


# revision 59
# speedup vs baseline: 1.0967x; 1.0205x over previous
"""Causal multi-head self-attention with RoPE on 8 Trainium2 NeuronCores.

Sharding: batch (4) x query-half (2) -> 8 cores, no collectives.
Each core computes full K/V for its batch; query rows are split between the
two cores of a batch in a causally-balanced schedule (4 slots of 256 rows
with 4/8/12/16 key-blocks each, ascending).  Causal masking is multiplicative
mask input data, so one SPMD program serves both halves.

Single fused pipeline, bf16 end-to-end (fp32 PSUM accumulation):
  per 512-seq chunk st: K^T proj+RoPE, V proj (+ones col), Q^T proj+RoPE,
  then attention slot st (which needs exactly k-blocks 0..4*(st+1)-1).
  Output projection for all slots is deferred to the end so it overlaps the
  ACT-bound tail of the last (deepest) attention slot, and so Wo can reuse
  Wk's SBUF space.

Layouts are transposed [feature, seq] so no on-device transposes are needed:
  K^T/Q^T = W^T.T @ X^T            (per 128-row head pair)
  RoPE    = cos*x + sin*(P@x)      (P = constant pair-rotation matrix)
  S^T     = Krot^T.T-slice @ Qrot^T  (keys on partitions; softmax along
                                      partitions via the ones-row trick)
  scores for both heads of a pair go into one [128,2,256] PSUM tile so a
  single Exp activation covers 512 elements (halves ACT instruction count)
  exp     = ACT Exp(scale=1/8) -> bf16
  A^T,l   = [V|1].T-free matmul accumulated over key blocks in PSUM
  denom broadcast = gpsimd partition_broadcast (keeps PE/ACT/DVE free)
  out     = A^T.T @ Wo^T           (natural [seq, feature] output layout)
"""

import os
import sys
import math

if "/opt/trn_rl_repo" not in sys.path:
    sys.path.append("/opt/trn_rl_repo")

import numpy as np
import ml_dtypes

import concourse.bass as bass
import concourse.tile as tile
from concourse import bacc, mybir
from concourse.bass_utils import run_bass_kernel_spmd

B = 4
S = 2048
D = 1024
H = 16
DK = 64
THETA = 10000.0

NEP = H // 2          # head pairs (128-partition groups)
QT = 256              # query tile width (free dim of score matmuls)
KB = 128              # key block (partition dim of score output)
NSLOT = 4             # query slots per core
CNT = [4, 8, 12, 16]  # k-blocks per slot (uniform across cores, ascending)
TILES_J = [[1, 3, 5, 7], [0, 2, 4, 6]]  # 256-row q-tile indices per half
VW = DK + 1           # V columns per head incl. trailing ones column

F32R = mybir.dt.float32r
F32 = mybir.dt.float32
BF16 = mybir.dt.bfloat16

_cache = {}

NO_GPSIMD = bool(int(os.environ.get("KERNEL_NO_GPSIMD", "1")))


def _build_program():
    if "nc" in _cache:
        return _cache["nc"]

    nc = bacc.Bacc("TRN2")

    xt_d = nc.dram_tensor("xt", [D, S], BF16, kind="ExternalInput")
    xq_d = nc.dram_tensor("xq", [D, NSLOT * QT], BF16, kind="ExternalInput")
    wkt_d = nc.dram_tensor("wkt", [D, D], BF16, kind="ExternalInput")
    wvt_d = nc.dram_tensor("wvt", [D, D], BF16, kind="ExternalInput")
    wqt_d = nc.dram_tensor("wqt", [D, D], BF16, kind="ExternalInput")
    wot_d = nc.dram_tensor("wot", [D, D], BF16, kind="ExternalInput")
    cosk_d = nc.dram_tensor("cosk", [128, S], BF16, kind="ExternalInput")
    sink_d = nc.dram_tensor("sink", [128, S], BF16, kind="ExternalInput")
    cosq_d = nc.dram_tensor("cosq", [128, NSLOT * QT], BF16, kind="ExternalInput")
    sinq_d = nc.dram_tensor("sinq", [128, NSLOT * QT], BF16, kind="ExternalInput")
    mask_d = nc.dram_tensor("mask", [128, 4, 2, QT], BF16, kind="ExternalInput")
    permt_d = nc.dram_tensor("permt", [128, 128], BF16, kind="ExternalInput")
    y_d = nc.dram_tensor("y", [NSLOT * QT, D], F32, kind="ExternalOutput")

    xt_t = xt_d.rearrange("(n p) s -> p n s", p=128)
    xq_t = xq_d.rearrange("(n p) s -> p n s", p=128)
    wkt_t = wkt_d.rearrange("(n p) e -> p n e", p=128)
    wqt_t = wqt_d.rearrange("(n p) e -> p n e", p=128)

    with tile.TileContext(nc) as tc:
        with (
            tc.tile_pool(name="wgt", bufs=1) as wgt,
            tc.tile_pool(name="kvq", bufs=1) as kvq,
            tc.tile_pool(name="tab", bufs=1) as tab,
            tc.tile_pool(name="xsp", bufs=2) as xsp,
            tc.tile_pool(name="xqp", bufs=1) as xqp,
            tc.tile_pool(name="wrk", bufs=2) as wrk,
            tc.tile_pool(name="epl", bufs=3) as epl,
            tc.tile_pool(name="atp", bufs=1) as atp,
            tc.tile_pool(name="nrm", bufs=2) as nrm,
            tc.tile_pool(name="ocp", bufs=2) as ocp,
            tc.tile_pool(name="ps", bufs=1, space="PSUM") as ps,
            tc.tile_pool(name="psa", bufs=2, space="PSUM") as psa,
        ):
            # ---- persistent SBUF tiles ----
            permt = wgt.tile([128, 128], BF16, tag="permt", bufs=1)
            wk = [wgt.tile([128, 8, 128], BF16, tag=f"wk{e}", bufs=1,
                           name=f"wk{e}") for e in range(NEP)]
            wq = [wgt.tile([128, 8, 128], BF16, tag=f"wq{e}", bufs=1,
                           name=f"wq{e}") for e in range(NEP)]
            wv = [wgt.tile([128, D], BF16, tag=f"wv{d}", bufs=1,
                           name=f"wv{d}") for d in range(8)]
            krot = [kvq.tile([128, S], BF16, tag=f"krot{e}", bufs=1,
                             name=f"krot{e}") for e in range(NEP)]
            vt = [kvq.tile([128, H * VW], BF16, tag=f"vt{k}", bufs=1,
                           name=f"vt{k}") for k in range(S // KB)]
            qrot = [kvq.tile([128, NSLOT * QT], BF16, tag=f"qrot{e}", bufs=1,
                             name=f"qrot{e}") for e in range(NEP)]
            cosk = tab.tile([128, S], BF16, tag="cosk", bufs=1)
            sink = tab.tile([128, S], BF16, tag="sink", bufs=1)
            cosq = tab.tile([128, NSLOT * QT], BF16, tag="cosq", bufs=1)
            sinq = tab.tile([128, NSLOT * QT], BF16, tag="sinq", bufs=1)
            masks = tab.tile([128, 4, 2, QT], BF16, tag="masks", bufs=1)

            # ---- input DMAs, in priority order ----
            for dd in range(4):
                nc.sync.dma_start(wk[0][:, 2 * dd:2 * dd + 2, :],
                                  wkt_t[:, 2 * dd:2 * dd + 2, 0:128])
            xs_first = xsp.tile([128, 8, 512], BF16, tag="xs", bufs=2,
                                name="xs_first")
            nc.gpsimd.dma_start(xs_first[:], xt_t[:, :, 0:512])
            xq_first = xqp.tile([128, 8, QT], BF16, tag="xq", bufs=1,
                                name="xq_first")
            nc.gpsimd.dma_start(xq_first[:], xq_t[:, :, 0:QT])
            for e in range(1, NEP):
                nc.sync.dma_start(wk[e][:], wkt_t[:, :, e * 128:(e + 1) * 128])
            nc.sync.dma_start(cosk[:], cosk_d[:])
            nc.sync.dma_start(sink[:], sink_d[:])
            nc.sync.dma_start(permt[:], permt_d[:])
            for d in range(8):
                nc.sync.dma_start(wv[d][:], wvt_d[d * 128:(d + 1) * 128, :])
            for e in range(NEP):
                nc.sync.dma_start(wq[e][:], wqt_t[:, :, e * 128:(e + 1) * 128])
            nc.sync.dma_start(cosq[:], cosq_d[:])
            nc.sync.dma_start(sinq[:], sinq_d[:])
            nc.sync.dma_start(masks[:], mask_d[:])

            # ones columns of vt (denominator rows for the AV matmul)
            if NO_GPSIMD:
                ones_t = tab.tile([VW, DK], F32, tag="ones", bufs=1)
                nc.vector.memset(ones_t[DK:VW, :], 1.0)
            for k in range(S // KB):
                nc.vector.memset(
                    vt[k].rearrange("p (h w) -> p h w", w=VW)[:, :, DK], 1.0
                )

            # ---- helpers ----
            def k_rope(kraw, e, csl):
                pp = ps.tile([128, 512], F32, tag="perm", bufs=1, name="ppk")
                nc.tensor.matmul(pp[:], permt[:], kraw[:], start=True, stop=True)
                t_c = wrk.tile([128, 512], BF16, tag="t_c", bufs=2, name="t_c")
                nc.vector.tensor_mul(t_c[:], kraw[:], cosk[:, csl])
                t_s = wrk.tile([128, 512], BF16, tag="t_s", bufs=2, name="t_s")
                nc.vector.tensor_mul(t_s[:], pp[:], sink[:, csl])
                nc.vector.tensor_add(krot[e][:, csl], t_c[:], t_s[:])

            def q_rope(qraw, e, csl):
                pp = ps.tile([128, QT], F32, tag="perm", bufs=1, name="ppq")
                nc.tensor.matmul(pp[:], permt[:], qraw[:], start=True, stop=True)
                t_c = wrk.tile([128, QT], BF16, tag="qt_c", bufs=2, name="qt_c")
                nc.vector.tensor_mul(t_c[:], qraw[:], cosq[:, csl])
                t_s = wrk.tile([128, QT], BF16, tag="qt_s", bufs=2, name="qt_s")
                nc.vector.tensor_mul(t_s[:], pp[:], sinq[:, csl])
                nc.vector.tensor_add(qrot[e][:, csl], t_c[:], t_s[:])

            aT = [[None] * NEP for _ in range(NSLOT)]

            def normalize(sl, e, acc):
                a = atp.tile([128, QT], BF16, tag=f"aT{sl}_{e}", bufs=1,
                             name=f"aT{sl}_{e}")
                aT[sl][e] = a
                lrow = nrm.tile([VW, 2, QT], F32R, tag="lrow", bufs=1,
                                name="lrow")
                with nc.allow_low_precision(
                    reason="f32r tile holds full f32 bits"
                ):
                    nc.vector.reciprocal(
                        lrow[DK:VW, :, :], acc[DK:VW, :, :]
                    )
                # move the reciprocal row to partition 0 (tiny SBUF DMA on
                # the idle gpsimd queue), then gpsimd-broadcast in place:
                # no PSUM bank, no DVE copy
                rb = nrm.tile([DK, 2, QT], F32R, tag="rb", bufs=1,
                              name="rb")
                nc.gpsimd.dma_start(rb[0:1, :, :], lrow[DK:VW, :, :])
                nc.gpsimd.partition_broadcast(rb[:], rb[0:1, :, :])
                nc.vector.tensor_mul(a[0:DK, :], acc[0:DK, 0, :],
                                     rb[:, 0, :])
                tmp = nrm.tile([DK, QT], BF16, tag="tmp", bufs=2,
                               name="tmp")
                nc.vector.tensor_mul(tmp[:], acc[0:DK, 1, :], rb[:, 1, :])
                nc.sync.dma_start(a[DK:128, :], tmp[:])

            def attention_pair(sl_a, sl_b, fillers=None):
                # slots sl_a, sl_b (= sl_a+1) are contiguous in qrot; for
                # k-blocks both need, ONE N=512 score matmul + ONE wide exp
                # covers both.  Cb-4 == Ca, so solo tiles are exactly the
                # masked tail of sl_b, and joint tiles only mask sl_a's tail.
                Ca, Cb = CNT[sl_a], CNT[sl_b]
                qsl_ab = slice(sl_a * QT, (sl_b + 1) * QT)
                qsl_b = slice(sl_b * QT, (sl_b + 1) * QT)
                for e in range(NEP):
                    acc_a = psa.tile([VW, 2, QT], F32, tag="acc", bufs=3,
                                     name="acc_a")
                    acc_b = psa.tile([VW, 2, QT], F32, tag="acc", bufs=3,
                                     name="acc_b")
                    pend = []

                    def flush_av(kb, exa, exb):
                        for h in range(2):
                            hh = 2 * e + h
                            vs = vt[kb][:, hh * VW:(hh + 1) * VW]
                            if exa is not None:
                                nc.tensor.matmul(
                                    acc_a[:, h, :], vs, exa[h],
                                    start=(kb == 0 and h == 0),
                                    stop=(kb == Ca - 1 and h == 1),
                                )
                            nc.tensor.matmul(
                                acc_b[:, h, :], vs, exb[h],
                                start=(kb == 0 and h == 0),
                                stop=(kb == Cb - 1 and h == 1),
                            )

                    for kb in range(Cb):
                        joint = kb < Ca
                        exa = [] if joint else None
                        exb = []
                        for h in range(2):
                            pb_ = h * DK
                            krs = krot[e][pb_:pb_ + DK,
                                          kb * KB:(kb + 1) * KB]
                            if joint:
                                psc = ps.tile([128, 2 * QT], F32, tag="sc",
                                              bufs=2, name="psc")
                                nc.tensor.matmul(
                                    psc[:], krs,
                                    qrot[e][pb_:pb_ + DK, qsl_ab],
                                    start=True, stop=True,
                                    tile_position=(pb_, 0),
                                )
                                ex = epl.tile([128, 2 * QT], BF16, tag="ex",
                                              bufs=5, name="ex")
                                nc.scalar.activation(
                                    ex[:], psc[:],
                                    mybir.ActivationFunctionType.Exp,
                                    scale=1.0 / math.sqrt(DK),
                                )
                                ea = ex[:, 0:QT]
                                if kb >= Ca - 4:
                                    em = epl.tile([128, QT], BF16, tag="em",
                                                  bufs=4, name="em")
                                    nc.vector.tensor_mul(
                                        em[:], ex[:, 0:QT],
                                        masks[:, kb - (Ca - 4), h, :]
                                    )
                                    ea = em[:]
                                exa.append(ea)
                                exb.append(ex[:, QT:2 * QT])
                            else:
                                psc = ps.tile([128, QT], F32, tag="sc",
                                              bufs=2, name="psc")
                                nc.tensor.matmul(
                                    psc[:], krs,
                                    qrot[e][pb_:pb_ + DK, qsl_b],
                                    start=True, stop=True,
                                    tile_position=(pb_, 0),
                                )
                                exh = epl.tile([128, QT], BF16, tag="exs",
                                               bufs=3, name="exh")
                                nc.scalar.activation(
                                    exh[:], psc[:],
                                    mybir.ActivationFunctionType.Exp,
                                    scale=1.0 / math.sqrt(DK),
                                )
                                em = epl.tile([128, QT], BF16, tag="em",
                                              bufs=4, name="em")
                                nc.vector.tensor_mul(
                                    em[:], exh[:],
                                    masks[:, kb - (Cb - 4), h, :]
                                )
                                exb.append(em[:])
                        pend.append((kb, exa, exb))
                        if len(pend) > 3:
                            flush_av(*pend.pop(0))
                    for p_ in pend:
                        flush_av(*p_)

                    normalize(sl_a, e, acc_a)
                    normalize(sl_b, e, acc_b)
                    if fillers and e >= 2:
                        for _ in range(2):
                            if fillers:
                                fillers.pop(0)()

            def oproj_group(sl, qs, et):
                def emit():
                    po = ps.tile([128, 512], F32, tag="proj", bufs=2,
                                 name="po")
                    for d in range(8):
                        nc.tensor.matmul(
                            po[:],
                            aT[sl][d][:, qs * 128:(qs + 1) * 128],
                            wo[d][:, et * 512:(et + 1) * 512],
                            start=(d == 0), stop=(d == 7),
                        )
                    for eh in range(2):
                        ot = ocp.tile([128, QT], F32, tag="ot", bufs=4,
                                      name="ot")
                        nc.vector.tensor_copy(ot[:], po[:, eh * QT:(eh + 1) * QT])
                        nc.sync.dma_start(
                            y_d[sl * QT + qs * 128:sl * QT + (qs + 1) * 128,
                                et * 512 + eh * QT:et * 512 + (eh + 1) * QT],
                            ot[:],
                        )
                return emit

            # ---- fused projection + attention pipeline ----
            xs_tiles = [xs_first]
            xq_tiles = [xq_first]
            for st in range(1, 4):
                # prefetches are issued at the top of each chunk below
                xs_tiles.append(None)
                xq_tiles.append(None)

            for st in range(4):
                if st > 0:
                    xs = xsp.tile([128, 8, 512], BF16, tag="xs", bufs=2,
                                  name="xs")
                    nc.gpsimd.dma_start(xs[:], xt_t[:, :, st * 512:(st + 1) * 512])
                    xq = xqp.tile([128, 8, QT], BF16, tag="xq", bufs=1,
                                  name="xq")
                    nc.gpsimd.dma_start(xq[:], xq_t[:, :, st * QT:(st + 1) * QT])
                else:
                    xs, xq = xs_first, xq_first

                # K projection + RoPE for seq columns [512*st, 512*(st+1))
                pend = []
                for e in range(NEP):
                    pk = ps.tile([128, 512], F32, tag="proj", bufs=2,
                                 name="pk")
                    for d in range(8):
                        nc.tensor.matmul(
                            pk[:], wk[e][:, d, :], xs[:, d, :],
                            start=(d == 0), stop=(d == 7),
                        )
                    kraw = wrk.tile([128, 512], BF16, tag="kraw", bufs=2,
                                    name="kraw")
                    nc.scalar.copy(kraw[:], pk[:])
                    pend.append((kraw, e, slice(st * 512, (st + 1) * 512)))
                    if len(pend) > 2:
                        k_rope(*pend.pop(0))

                # V projection for k-blocks 4*st .. 4*st+3
                for half in range(4):
                    kb = 4 * st + half
                    off = half * KB
                    for et in range(2):
                        pv = ps.tile([128, 512], F32, tag="proj", bufs=2,
                                     name="pv")
                        for d in range(8):
                            nc.tensor.matmul(
                                pv[:], xs[:, d, off:off + KB],
                                wv[d][:, et * 512:(et + 1) * 512],
                                start=(d == 0), stop=(d == 7),
                            )
                        dst = vt[kb].rearrange("p (h w) -> p h w", w=VW)
                        if et == 0:
                            nc.scalar.copy(
                                dst[:, 0:8, 0:DK],
                                pv[:].rearrange("p (h w) -> p h w", w=DK),
                            )
                        else:
                            nc.scalar.copy(
                                dst[:, 8:16, 0:DK],
                                pv[:].rearrange("p (h w) -> p h w", w=DK),
                            )
                    if len(pend) > 1:
                        k_rope(*pend.pop(0))

                # Q projection + RoPE for slot st
                pendq = []
                for e in range(NEP):
                    pq = ps.tile([128, QT], F32, tag="proj", bufs=2,
                                 name="pq")
                    for d in range(8):
                        nc.tensor.matmul(
                            pq[:], wq[e][:, d, :], xq[:, d, :],
                            start=(d == 0), stop=(d == 7),
                        )
                    qraw = wrk.tile([128, QT], BF16, tag="qraw", bufs=2,
                                    name="qraw")
                    nc.vector.tensor_copy(qraw[:], pq[:])
                    pendq.append((qraw, e, slice(st * QT, (st + 1) * QT)))
                    if len(pendq) > 2:
                        q_rope(*pendq.pop(0))
                for p_ in pend:
                    k_rope(*p_)
                for p_ in pendq:
                    q_rope(*p_)

                if st == 3:
                    # Wo reuses Wk's SBUF slots (tag ring); DMA starts once
                    # chunk 3's K projection has consumed wk, and overlaps
                    # the deepest attention slot below.
                    wo = [wgt.tile([128, D], BF16, tag=f"wk{e}", bufs=1,
                                   name=f"wo{e}") for e in range(NEP)]
                    for e in range(NEP):
                        nc.sync.dma_start(wo[e][:],
                                          wot_d[e * 128:(e + 1) * 128, :])

                if st == 1:
                    attention_pair(0, 1)
                elif st == 3:
                    # slots 0/1's output projections fill PE while the deep
                    # pair's softmax keeps the scalar engine saturated
                    fillers = [oproj_group(sl, qs, et)
                               for sl in range(2)
                               for qs in range(2) for et in range(2)]
                    attention_pair(2, 3, fillers)
                    for f in fillers:
                        f()

            # ---- slots 2/3 output projection ----
            for sl in (2, 3):
                for qs in range(2):
                    for et in range(2):
                        oproj_group(sl, qs, et)()

    nc.compile()
    nc.finalize()
    _cache["nc"] = nc
    return nc


def _rope_tables(pos):
    """cos/sin tables in [128, n] head-pair layout (row e -> pair (e%64)//2)."""
    k = np.arange(DK // 2, dtype=np.float32)
    inv_freq = (THETA ** (-2.0 * k / DK)).astype(np.float32)
    ang = inv_freq[:, None] * pos.astype(np.float32)[None, :]  # [32, n]
    cos64 = np.repeat(np.cos(ang), 2, axis=0)
    sin64 = np.repeat(np.sin(ang), 2, axis=0)
    cos = np.concatenate([cos64, cos64], axis=0)
    sin = np.concatenate([sin64, sin64], axis=0)
    return (np.ascontiguousarray(cos).astype(ml_dtypes.bfloat16),
            np.ascontiguousarray(sin).astype(ml_dtypes.bfloat16))


def _masks(j):
    """[128, 4, 2, QT] bf16 multiplicative causal masks for half j.

    Slot-independent: for slot sl (C = CNT[sl]) the mask applies to the last
    four k-blocks C-4..C-1.  j=0 owns tiles 2t+1 -> [1, 1, triA, triB];
    j=1 owns tiles 2t -> [triA, triB, 0, 0].
    """
    p = np.arange(KB)[:, None]
    f = np.arange(QT)[None, :]
    triA = (f >= p).astype(np.float32)
    triB = (f >= p + KB).astype(np.float32)
    ones = np.ones((KB, QT), np.float32)
    zeros = np.zeros((KB, QT), np.float32)
    blocks = [ones, ones, triA, triB] if j == 0 else [triA, triB, zeros, zeros]
    m = np.stack([np.stack([blk] * 2, axis=0) for blk in blocks], axis=0)
    # [4, 2, KB, QT] -> [KB, 4, 2, QT]
    return np.ascontiguousarray(m.transpose(2, 0, 1, 3)).astype(
        ml_dtypes.bfloat16)


def _host_inputs(in_features, token_positions, Wq, Wk, Wv, Wo):
    X = np.asarray(in_features, dtype=np.float32)
    pos = np.asarray(token_positions)
    bf = ml_dtypes.bfloat16
    wqt = np.ascontiguousarray(np.asarray(Wq, np.float32).T).astype(bf)
    wkt = np.ascontiguousarray(np.asarray(Wk, np.float32).T).astype(bf)
    wvt = np.ascontiguousarray(np.asarray(Wv, np.float32).T).astype(bf)
    wot = np.ascontiguousarray(np.asarray(Wo, np.float32).T).astype(bf)
    cosk, sink = _rope_tables(pos)

    permt = np.zeros((128, 128), np.float32)
    for i in range(64):
        permt[2 * i + 1, 2 * i] = -1.0
        permt[2 * i, 2 * i + 1] = 1.0
    permt = permt.astype(bf)

    in_maps = []
    for core in range(8):
        b, j = core // 2, core % 2
        rows = np.concatenate(
            [np.arange(t * QT, (t + 1) * QT) for t in TILES_J[j]])
        cosq, sinq = _rope_tables(pos[rows])
        in_maps.append({
            "xt": np.ascontiguousarray(X[b].T).astype(bf),
            "xq": np.ascontiguousarray(X[b][rows].T).astype(bf),
            "wkt": wkt, "wvt": wvt, "wqt": wqt, "wot": wot,
            "cosk": cosk, "sink": sink, "cosq": cosq, "sinq": sinq,
            "mask": _masks(j), "permt": permt,
        })
    return in_maps


def kernel(in_features, token_positions, Wq, Wk, Wv, Wo):
    nc = _build_program()
    in_maps = _host_inputs(in_features, token_positions, Wq, Wk, Wv, Wo)

    trace = bool(int(os.environ.get("KERNEL_TRACE", "0")))
    res = run_bass_kernel_spmd(nc, in_maps, core_ids=list(range(8)), trace=trace)
    kernel.last_result = res

    out = np.empty((B, S, D), np.float32)
    for core in range(8):
        b, j = core // 2, core % 2
        y = res.results[core]["y"]
        for s_i, t in enumerate(TILES_J[j]):
            out[b, t * QT:(t + 1) * QT, :] = y[s_i * QT:(s_i + 1) * QT, :]
    return out


# revision 68
# speedup vs baseline: 1.1011x; 1.0041x over previous
"""Causal multi-head self-attention with RoPE on 8 Trainium2 NeuronCores.

Sharding: batch (4) x query-half (2) -> 8 cores, no collectives.
Each core computes full K/V for its batch; query rows are split between the
two cores of a batch in a causally-balanced schedule (4 slots of 256 rows
with 4/8/12/16 key-blocks each, ascending).  Causal masking is multiplicative
mask input data, so one SPMD program serves both halves.

Single fused pipeline, bf16 end-to-end (fp32 PSUM accumulation):
  per 512-seq chunk st: K^T proj+RoPE, V proj (+ones col), Q^T proj+RoPE,
  then attention slot st (which needs exactly k-blocks 0..4*(st+1)-1).
  Output projection for all slots is deferred to the end so it overlaps the
  ACT-bound tail of the last (deepest) attention slot, and so Wo can reuse
  Wk's SBUF space.

Layouts are transposed [feature, seq] so no on-device transposes are needed:
  K^T/Q^T = W^T.T @ X^T            (per 128-row head pair)
  RoPE    = cos*x + sin*(P@x)      (P = constant pair-rotation matrix)
  S^T     = Krot^T.T-slice @ Qrot^T  (keys on partitions; softmax along
                                      partitions via the ones-row trick)
  scores for both heads of a pair go into one [128,2,256] PSUM tile so a
  single Exp activation covers 512 elements (halves ACT instruction count)
  exp     = ACT Exp(scale=1/8) -> bf16
  A^T,l   = [V|1].T-free matmul accumulated over key blocks in PSUM
  denom broadcast = gpsimd partition_broadcast (keeps PE/ACT/DVE free)
  out     = A^T.T @ Wo^T           (natural [seq, feature] output layout)
"""

import os
import sys
import math

if "/opt/trn_rl_repo" not in sys.path:
    sys.path.append("/opt/trn_rl_repo")

import numpy as np
import ml_dtypes

import concourse.bass as bass
import concourse.tile as tile
from concourse import bacc, mybir
from concourse.bass_utils import run_bass_kernel_spmd

B = 4
S = 2048
D = 1024
H = 16
DK = 64
THETA = 10000.0

NEP = H // 2          # head pairs (128-partition groups)
QT = 256              # query tile width (free dim of score matmuls)
KB = 128              # key block (partition dim of score output)
NSLOT = 4             # query slots per core
CNT = [4, 8, 12, 16]  # k-blocks per slot (uniform across cores, ascending)
TILES_J = [[1, 3, 5, 7], [0, 2, 4, 6]]  # 256-row q-tile indices per half
VW = DK + 1           # V columns per head incl. trailing ones column

F32R = mybir.dt.float32r
F32 = mybir.dt.float32
BF16 = mybir.dt.bfloat16

_cache = {}

NO_GPSIMD = bool(int(os.environ.get("KERNEL_NO_GPSIMD", "1")))


def _build_program():
    if "nc" in _cache:
        return _cache["nc"]

    nc = bacc.Bacc("TRN2")

    xt_d = nc.dram_tensor("xt", [D, S], BF16, kind="ExternalInput")
    xq_d = nc.dram_tensor("xq", [D, NSLOT * QT], BF16, kind="ExternalInput")
    wkt_d = nc.dram_tensor("wkt", [D, D], BF16, kind="ExternalInput")
    wvt_d = nc.dram_tensor("wvt", [D, D], BF16, kind="ExternalInput")
    wqt_d = nc.dram_tensor("wqt", [D, D], BF16, kind="ExternalInput")
    wot_d = nc.dram_tensor("wot", [D, D], BF16, kind="ExternalInput")
    cosk_d = nc.dram_tensor("cosk", [128, S], BF16, kind="ExternalInput")
    sink_d = nc.dram_tensor("sink", [128, S], BF16, kind="ExternalInput")
    cosq_d = nc.dram_tensor("cosq", [128, NSLOT * QT], BF16, kind="ExternalInput")
    sinq_d = nc.dram_tensor("sinq", [128, NSLOT * QT], BF16, kind="ExternalInput")
    mask_d = nc.dram_tensor("mask", [128, 4, 2, QT], BF16, kind="ExternalInput")
    permt_d = nc.dram_tensor("permt", [128, 128], BF16, kind="ExternalInput")
    y_d = nc.dram_tensor("y", [NSLOT * QT, D], F32, kind="ExternalOutput")

    xt_t = xt_d.rearrange("(n p) s -> p n s", p=128)
    xq_t = xq_d.rearrange("(n p) s -> p n s", p=128)
    wkt_t = wkt_d.rearrange("(n p) e -> p n e", p=128)
    wqt_t = wqt_d.rearrange("(n p) e -> p n e", p=128)

    with tile.TileContext(nc) as tc:
        with (
            tc.tile_pool(name="wgt", bufs=1) as wgt,
            tc.tile_pool(name="kvq", bufs=1) as kvq,
            tc.tile_pool(name="tab", bufs=1) as tab,
            tc.tile_pool(name="xsp", bufs=2) as xsp,
            tc.tile_pool(name="xqp", bufs=1) as xqp,
            tc.tile_pool(name="wrk", bufs=2) as wrk,
            tc.tile_pool(name="epl", bufs=3) as epl,
            tc.tile_pool(name="atp", bufs=1) as atp,
            tc.tile_pool(name="nrm", bufs=2) as nrm,
            tc.tile_pool(name="ocp", bufs=2) as ocp,
            tc.tile_pool(name="ps", bufs=1, space="PSUM") as ps,
            tc.tile_pool(name="psa", bufs=2, space="PSUM") as psa,
        ):
            # ---- persistent SBUF tiles ----
            permt = wgt.tile([128, 128], BF16, tag="permt", bufs=1)
            wk = [wgt.tile([128, 8, 128], BF16, tag=f"wk{e}", bufs=1,
                           name=f"wk{e}") for e in range(NEP)]
            wq = [wgt.tile([128, 8, 128], BF16, tag=f"wq{e}", bufs=1,
                           name=f"wq{e}") for e in range(NEP)]
            wv = [wgt.tile([128, D], BF16, tag=f"wv{d}", bufs=1,
                           name=f"wv{d}") for d in range(8)]
            krot = [kvq.tile([128, S], BF16, tag=f"krot{e}", bufs=1,
                             name=f"krot{e}") for e in range(NEP)]
            vt = [kvq.tile([128, H * VW], BF16, tag=f"vt{k}", bufs=1,
                           name=f"vt{k}") for k in range(S // KB)]
            qrot = [kvq.tile([128, NSLOT * QT], BF16, tag=f"qrot{e}", bufs=1,
                             name=f"qrot{e}") for e in range(NEP)]
            cosk = tab.tile([128, S], BF16, tag="cosk", bufs=1)
            sink = tab.tile([128, S], BF16, tag="sink", bufs=1)
            cosq = tab.tile([128, NSLOT * QT], BF16, tag="cosq", bufs=1)
            sinq = tab.tile([128, NSLOT * QT], BF16, tag="sinq", bufs=1)
            masks = tab.tile([128, 4, 2, QT], BF16, tag="masks", bufs=1)

            # ---- input DMAs, in priority order ----
            for dd in range(4):
                nc.sync.dma_start(wk[0][:, 2 * dd:2 * dd + 2, :],
                                  wkt_t[:, 2 * dd:2 * dd + 2, 0:128])
            xs_first = xsp.tile([128, 8, 512], BF16, tag="xs", bufs=2,
                                name="xs_first")
            nc.gpsimd.dma_start(xs_first[:], xt_t[:, :, 0:512])
            xq_first = xqp.tile([128, 8, QT], BF16, tag="xq", bufs=1,
                                name="xq_first")
            nc.gpsimd.dma_start(xq_first[:], xq_t[:, :, 0:QT])
            for e in range(1, NEP):
                nc.sync.dma_start(wk[e][:], wkt_t[:, :, e * 128:(e + 1) * 128])
            nc.sync.dma_start(cosk[:], cosk_d[:])
            nc.sync.dma_start(sink[:], sink_d[:])
            nc.sync.dma_start(permt[:], permt_d[:])
            for d in range(8):
                nc.sync.dma_start(wv[d][:], wvt_d[d * 128:(d + 1) * 128, :])
            for e in range(NEP):
                nc.sync.dma_start(wq[e][:], wqt_t[:, :, e * 128:(e + 1) * 128])
            nc.sync.dma_start(cosq[:], cosq_d[:])
            nc.sync.dma_start(sinq[:], sinq_d[:])
            nc.sync.dma_start(masks[:], mask_d[:])

            # ones columns of vt (denominator rows for the AV matmul)
            if NO_GPSIMD:
                ones_t = tab.tile([VW, DK], F32, tag="ones", bufs=1)
                nc.vector.memset(ones_t[DK:VW, :], 1.0)
            for k in range(S // KB):
                nc.vector.memset(
                    vt[k].rearrange("p (h w) -> p h w", w=VW)[:, :, DK], 1.0
                )

            # ---- helpers ----
            def k_rope(kraw, e, csl):
                pp = ps.tile([128, 512], F32, tag="perm", bufs=1, name="ppk")
                nc.tensor.matmul(pp[:], permt[:], kraw[:], start=True, stop=True)
                t_c = wrk.tile([128, 512], BF16, tag="t_c", bufs=2, name="t_c")
                nc.vector.tensor_mul(t_c[:], kraw[:], cosk[:, csl])
                t_s = wrk.tile([128, 512], BF16, tag="t_s", bufs=2, name="t_s")
                nc.vector.tensor_mul(t_s[:], pp[:], sink[:, csl])
                nc.vector.tensor_add(krot[e][:, csl], t_c[:], t_s[:])

            def q_rope(qraw, e, csl):
                pp = ps.tile([128, QT], F32, tag="perm", bufs=1, name="ppq")
                nc.tensor.matmul(pp[:], permt[:], qraw[:], start=True, stop=True)
                t_c = wrk.tile([128, QT], BF16, tag="qt_c", bufs=2, name="qt_c")
                nc.vector.tensor_mul(t_c[:], qraw[:], cosq[:, csl])
                t_s = wrk.tile([128, QT], BF16, tag="qt_s", bufs=2, name="qt_s")
                nc.vector.tensor_mul(t_s[:], pp[:], sinq[:, csl])
                nc.vector.tensor_add(qrot[e][:, csl], t_c[:], t_s[:])

            aT = [[None] * NEP for _ in range(NSLOT)]

            def normalize(sl, e, acc):
                a = atp.tile([128, QT], BF16, tag=f"aT{sl}_{e}", bufs=1,
                             name=f"aT{sl}_{e}")
                aT[sl][e] = a
                lrow = nrm.tile([VW, 2, QT], F32R, tag="lrow", bufs=1,
                                name="lrow")
                with nc.allow_low_precision(
                    reason="f32r tile holds full f32 bits"
                ):
                    nc.vector.reciprocal(
                        lrow[DK:VW, :, :], acc[DK:VW, :, :]
                    )
                if e == NEP - 1:
                    # final ep: the shorter matmul-broadcast chain (sc ring
                    # is idling out) keeps the output projection tail tight
                    pb = ps.tile([DK, 2, QT], F32, tag="sc", bufs=2,
                                 name="pb")
                    nc.tensor.matmul(pb[:], ones_t.bitcast(F32R)[DK:VW, :],
                                     lrow[DK:VW, :, :], start=True, stop=True)
                    rb = nrm.tile([DK, 2, QT], F32, tag="rb", bufs=1,
                                  name="rbf")
                    nc.vector.tensor_copy(rb[:], pb[:])
                else:
                    # move the reciprocal row to partition 0 (tiny SBUF DMA
                    # on the idle gpsimd queue), then gpsimd-broadcast in
                    # place: no PSUM bank, no DVE copy
                    rb = nrm.tile([DK, 2, QT], F32R, tag="rb", bufs=1,
                                  name="rb")
                    nc.gpsimd.dma_start(rb[0:1, :, :], lrow[DK:VW, :, :])
                    nc.gpsimd.partition_broadcast(rb[:], rb[0:1, :, :])
                nc.vector.tensor_mul(a[0:DK, :], acc[0:DK, 0, :],
                                     rb[:, 0, :])
                tmp = nrm.tile([DK, QT], BF16, tag="tmp", bufs=2,
                               name="tmp")
                nc.vector.tensor_mul(tmp[:], acc[0:DK, 1, :], rb[:, 1, :])
                nc.sync.dma_start(a[DK:128, :], tmp[:])

            def attention_pair(sl_a, sl_b, fillers=None):
                # slots sl_a, sl_b (= sl_a+1) are contiguous in qrot; for
                # k-blocks both need, ONE N=512 score matmul + ONE wide exp
                # covers both.  Cb-4 == Ca, so solo tiles are exactly the
                # masked tail of sl_b, and joint tiles only mask sl_a's tail.
                Ca, Cb = CNT[sl_a], CNT[sl_b]
                qsl_ab = slice(sl_a * QT, (sl_b + 1) * QT)
                qsl_b = slice(sl_b * QT, (sl_b + 1) * QT)
                for e in range(NEP):
                    acc_a = psa.tile([VW, 2, QT], F32, tag="acc", bufs=3,
                                     name="acc_a")
                    acc_b = psa.tile([VW, 2, QT], F32, tag="acc", bufs=3,
                                     name="acc_b")
                    pend = []

                    def flush_av(kb, exa, exb):
                        for h in range(2):
                            hh = 2 * e + h
                            vs = vt[kb][:, hh * VW:(hh + 1) * VW]
                            if exa is not None:
                                nc.tensor.matmul(
                                    acc_a[:, h, :], vs, exa[h],
                                    start=(kb == 0 and h == 0),
                                    stop=(kb == Ca - 1 and h == 1),
                                )
                            nc.tensor.matmul(
                                acc_b[:, h, :], vs, exb[h],
                                start=(kb == 0 and h == 0),
                                stop=(kb == Cb - 1 and h == 1),
                            )

                    for kb in range(Cb):
                        joint = kb < Ca
                        exa = [] if joint else None
                        exb = []
                        for h in range(2):
                            pb_ = h * DK
                            krs = krot[e][pb_:pb_ + DK,
                                          kb * KB:(kb + 1) * KB]
                            if joint:
                                psc = ps.tile([128, 2 * QT], F32, tag="sc",
                                              bufs=2, name="psc")
                                nc.tensor.matmul(
                                    psc[:], krs,
                                    qrot[e][pb_:pb_ + DK, qsl_ab],
                                    start=True, stop=True,
                                    tile_position=(pb_, 0),
                                )
                                ex = epl.tile([128, 2 * QT], BF16, tag="ex",
                                              bufs=5, name="ex")
                                nc.scalar.activation(
                                    ex[:], psc[:],
                                    mybir.ActivationFunctionType.Exp,
                                    scale=1.0 / math.sqrt(DK),
                                )
                                ea = ex[:, 0:QT]
                                if kb >= Ca - 4:
                                    em = epl.tile([128, QT], BF16, tag="em",
                                                  bufs=4, name="em")
                                    nc.vector.tensor_mul(
                                        em[:], ex[:, 0:QT],
                                        masks[:, kb - (Ca - 4), h, :]
                                    )
                                    ea = em[:]
                                exa.append(ea)
                                exb.append(ex[:, QT:2 * QT])
                            else:
                                psc = ps.tile([128, QT], F32, tag="sc",
                                              bufs=2, name="psc")
                                nc.tensor.matmul(
                                    psc[:], krs,
                                    qrot[e][pb_:pb_ + DK, qsl_b],
                                    start=True, stop=True,
                                    tile_position=(pb_, 0),
                                )
                                exh = epl.tile([128, QT], BF16, tag="exs",
                                               bufs=3, name="exh")
                                nc.scalar.activation(
                                    exh[:], psc[:],
                                    mybir.ActivationFunctionType.Exp,
                                    scale=1.0 / math.sqrt(DK),
                                )
                                em = epl.tile([128, QT], BF16, tag="em",
                                              bufs=4, name="em")
                                nc.vector.tensor_mul(
                                    em[:], exh[:],
                                    masks[:, kb - (Cb - 4), h, :]
                                )
                                exb.append(em[:])
                        pend.append((kb, exa, exb))
                        if len(pend) > 3:
                            flush_av(*pend.pop(0))
                    for p_ in pend:
                        flush_av(*p_)

                    normalize(sl_a, e, acc_a)
                    normalize(sl_b, e, acc_b)
                    if fillers and e >= 2:
                        for _ in range(2):
                            if fillers:
                                fillers.pop(0)()

            def oproj_group(sl, qs, et):
                def emit():
                    po = ps.tile([128, 512], F32, tag="proj", bufs=2,
                                 name="po")
                    for d in range(8):
                        nc.tensor.matmul(
                            po[:],
                            aT[sl][d][:, qs * 128:(qs + 1) * 128],
                            wo[d][:, et * 512:(et + 1) * 512],
                            start=(d == 0), stop=(d == 7),
                        )
                    for eh in range(2):
                        ot = ocp.tile([128, QT], F32, tag="ot", bufs=4,
                                      name="ot")
                        nc.vector.tensor_copy(ot[:], po[:, eh * QT:(eh + 1) * QT])
                        nc.sync.dma_start(
                            y_d[sl * QT + qs * 128:sl * QT + (qs + 1) * 128,
                                et * 512 + eh * QT:et * 512 + (eh + 1) * QT],
                            ot[:],
                        )
                return emit

            # ---- fused projection + attention pipeline ----
            xs_tiles = [xs_first]
            xq_tiles = [xq_first]
            for st in range(1, 4):
                # prefetches are issued at the top of each chunk below
                xs_tiles.append(None)
                xq_tiles.append(None)

            for st in range(4):
                if st > 0:
                    xs = xsp.tile([128, 8, 512], BF16, tag="xs", bufs=2,
                                  name="xs")
                    nc.gpsimd.dma_start(xs[:], xt_t[:, :, st * 512:(st + 1) * 512])
                    xq = xqp.tile([128, 8, QT], BF16, tag="xq", bufs=1,
                                  name="xq")
                    nc.gpsimd.dma_start(xq[:], xq_t[:, :, st * QT:(st + 1) * QT])
                else:
                    xs, xq = xs_first, xq_first

                # K projection + RoPE for seq columns [512*st, 512*(st+1))
                pend = []
                for e in range(NEP):
                    pk = ps.tile([128, 512], F32, tag="proj", bufs=2,
                                 name="pk")
                    for d in range(8):
                        nc.tensor.matmul(
                            pk[:], wk[e][:, d, :], xs[:, d, :],
                            start=(d == 0), stop=(d == 7),
                        )
                    kraw = wrk.tile([128, 512], BF16, tag="kraw", bufs=2,
                                    name="kraw")
                    nc.scalar.copy(kraw[:], pk[:])
                    pend.append((kraw, e, slice(st * 512, (st + 1) * 512)))
                    if len(pend) > 2:
                        k_rope(*pend.pop(0))

                # V projection for k-blocks 4*st .. 4*st+3
                for half in range(4):
                    kb = 4 * st + half
                    off = half * KB
                    for et in range(2):
                        pv = ps.tile([128, 512], F32, tag="proj", bufs=2,
                                     name="pv")
                        for d in range(8):
                            nc.tensor.matmul(
                                pv[:], xs[:, d, off:off + KB],
                                wv[d][:, et * 512:(et + 1) * 512],
                                start=(d == 0), stop=(d == 7),
                            )
                        dst = vt[kb].rearrange("p (h w) -> p h w", w=VW)
                        if et == 0:
                            nc.scalar.copy(
                                dst[:, 0:8, 0:DK],
                                pv[:].rearrange("p (h w) -> p h w", w=DK),
                            )
                        else:
                            nc.scalar.copy(
                                dst[:, 8:16, 0:DK],
                                pv[:].rearrange("p (h w) -> p h w", w=DK),
                            )
                    if len(pend) > 1:
                        k_rope(*pend.pop(0))

                # Q projection + RoPE for slot st
                pendq = []
                for e in range(NEP):
                    pq = ps.tile([128, QT], F32, tag="proj", bufs=2,
                                 name="pq")
                    for d in range(8):
                        nc.tensor.matmul(
                            pq[:], wq[e][:, d, :], xq[:, d, :],
                            start=(d == 0), stop=(d == 7),
                        )
                    qraw = wrk.tile([128, QT], BF16, tag="qraw", bufs=2,
                                    name="qraw")
                    nc.vector.tensor_copy(qraw[:], pq[:])
                    pendq.append((qraw, e, slice(st * QT, (st + 1) * QT)))
                    if len(pendq) > 2:
                        q_rope(*pendq.pop(0))
                for p_ in pend:
                    k_rope(*p_)
                for p_ in pendq:
                    q_rope(*p_)

                if st == 3:
                    # Wo reuses Wk's SBUF slots (tag ring); DMA starts once
                    # chunk 3's K projection has consumed wk, and overlaps
                    # the deepest attention slot below.
                    wo = [wgt.tile([128, D], BF16, tag=f"wk{e}", bufs=1,
                                   name=f"wo{e}") for e in range(NEP)]
                    for e in range(NEP):
                        nc.sync.dma_start(wo[e][:],
                                          wot_d[e * 128:(e + 1) * 128, :])

                if st == 1:
                    attention_pair(0, 1)
                elif st == 3:
                    # slots 0/1's output projections fill PE while the deep
                    # pair's softmax keeps the scalar engine saturated
                    fillers = [oproj_group(sl, qs, et)
                               for sl in range(2)
                               for qs in range(2) for et in range(2)]
                    attention_pair(2, 3, fillers)
                    for f in fillers:
                        f()

            # ---- slots 2/3 output projection ----
            for sl in (2, 3):
                for qs in range(2):
                    for et in range(2):
                        oproj_group(sl, qs, et)()

    nc.compile()
    nc.finalize()
    _cache["nc"] = nc
    return nc


def _rope_tables(pos):
    """cos/sin tables in [128, n] head-pair layout (row e -> pair (e%64)//2)."""
    k = np.arange(DK // 2, dtype=np.float32)
    inv_freq = (THETA ** (-2.0 * k / DK)).astype(np.float32)
    ang = inv_freq[:, None] * pos.astype(np.float32)[None, :]  # [32, n]
    cos64 = np.repeat(np.cos(ang), 2, axis=0)
    sin64 = np.repeat(np.sin(ang), 2, axis=0)
    cos = np.concatenate([cos64, cos64], axis=0)
    sin = np.concatenate([sin64, sin64], axis=0)
    return (np.ascontiguousarray(cos).astype(ml_dtypes.bfloat16),
            np.ascontiguousarray(sin).astype(ml_dtypes.bfloat16))


def _masks(j):
    """[128, 4, 2, QT] bf16 multiplicative causal masks for half j.

    Slot-independent: for slot sl (C = CNT[sl]) the mask applies to the last
    four k-blocks C-4..C-1.  j=0 owns tiles 2t+1 -> [1, 1, triA, triB];
    j=1 owns tiles 2t -> [triA, triB, 0, 0].
    """
    p = np.arange(KB)[:, None]
    f = np.arange(QT)[None, :]
    triA = (f >= p).astype(np.float32)
    triB = (f >= p + KB).astype(np.float32)
    ones = np.ones((KB, QT), np.float32)
    zeros = np.zeros((KB, QT), np.float32)
    blocks = [ones, ones, triA, triB] if j == 0 else [triA, triB, zeros, zeros]
    m = np.stack([np.stack([blk] * 2, axis=0) for blk in blocks], axis=0)
    # [4, 2, KB, QT] -> [KB, 4, 2, QT]
    return np.ascontiguousarray(m.transpose(2, 0, 1, 3)).astype(
        ml_dtypes.bfloat16)


def _host_inputs(in_features, token_positions, Wq, Wk, Wv, Wo):
    X = np.asarray(in_features, dtype=np.float32)
    pos = np.asarray(token_positions)
    bf = ml_dtypes.bfloat16
    wqt = np.ascontiguousarray(np.asarray(Wq, np.float32).T).astype(bf)
    wkt = np.ascontiguousarray(np.asarray(Wk, np.float32).T).astype(bf)
    wvt = np.ascontiguousarray(np.asarray(Wv, np.float32).T).astype(bf)
    wot = np.ascontiguousarray(np.asarray(Wo, np.float32).T).astype(bf)
    cosk, sink = _rope_tables(pos)

    permt = np.zeros((128, 128), np.float32)
    for i in range(64):
        permt[2 * i + 1, 2 * i] = -1.0
        permt[2 * i, 2 * i + 1] = 1.0
    permt = permt.astype(bf)

    in_maps = []
    for core in range(8):
        b, j = core // 2, core % 2
        rows = np.concatenate(
            [np.arange(t * QT, (t + 1) * QT) for t in TILES_J[j]])
        cosq, sinq = _rope_tables(pos[rows])
        in_maps.append({
            "xt": np.ascontiguousarray(X[b].T).astype(bf),
            "xq": np.ascontiguousarray(X[b][rows].T).astype(bf),
            "wkt": wkt, "wvt": wvt, "wqt": wqt, "wot": wot,
            "cosk": cosk, "sink": sink, "cosq": cosq, "sinq": sinq,
            "mask": _masks(j), "permt": permt,
        })
    return in_maps


def kernel(in_features, token_positions, Wq, Wk, Wv, Wo):
    nc = _build_program()
    in_maps = _host_inputs(in_features, token_positions, Wq, Wk, Wv, Wo)

    trace = bool(int(os.environ.get("KERNEL_TRACE", "0")))
    res = run_bass_kernel_spmd(nc, in_maps, core_ids=list(range(8)), trace=trace)
    kernel.last_result = res

    out = np.empty((B, S, D), np.float32)
    for core in range(8):
        b, j = core // 2, core % 2
        y = res.results[core]["y"]
        for s_i, t in enumerate(TILES_J[j]):
            out[b, t * QT:(t + 1) * QT, :] = y[s_i * QT:(s_i + 1) * QT, :]
    return out
